# revision 48
# baseline (speedup 1.0000x reference)
"""Trainium2 Bass kernel for nn_CRF_SelfAttention_65627100283470.

Math (validated vs the reference at 1e-6 rel err):
  - The CRF/marginal branch is dead code: softmax over the class dim sums
    to 1, so sum(cluster_features, 0) == sum of context rows.  The output
    is (sum_{f,p} context2) @ cls_W + cls_b.
  - context2 = w2*T2 + w1*(1-w2)*T1 with T_it the per-iteration temporal
    tensors, and w_it per-frame halting weights -> only per-frame sums of
    temporal are needed at the end.
  - QKV projections are shared across overlapping windows; exp(scores)
    strips are shared across windows (computed per key-frame strip); the
    output projection commutes with overlap-add; softmax denominators come
    from a ones-column prepended to V.
  - The 1/overlap-count scaling commutes with the A@V drain (it is a
    per-(scale, query-frame) scalar), so it is applied once per piece to
    abarT (iter 1) / once to asum (iter 2) instead of per window.

Sharding: 8 heads -> 8 cores.  The mid-kernel temporal exchange is TWO
chunked AllReduces (pieces 0-2 fired mid-s=6-sweep, pieces 3-5 at the
end) whose copy-backs issue from the ACT queue right before the iter-2
QKV that consumes them; iter-2's head is emitted in two piece-halves so
chunk-1 work hides chunk-2's collective.  The final collective reduces
the [1,625] class output itself (coefs/ssum0 are replicated).

This revision (vs the 383us v1), all bf16 on the PE:
  - abar is bf16 with a constant-ones column per frame; Wo is a 97-row
    matrix whose last row carries bo.sum, folding the bias into the
    matmul so both Wo drains are single scaled copies on ACT.
  - Wo projection + AllReduce staging run per 3-frame piece, pipelined
    one group behind the s=6 A@V sweep (PE never head-blocks on the
    drain/transpose chain).
  - iter-2 frame sums via tiny PE column-sum matmuls (no DMA transposes)
    and the halting/coef chains hoisted to the iteration head.
  - drain: one batched reciprocal per group (132-pitch transposed
    tiles); overlap-add accumulation on gpsimd; overlap-count scaling
    applied once per piece (abarT) / once to asum instead of per window.

Measured dead ends (do not revisit without new evidence): fp8e4
DoubleRow A@V is ~30% SLOWER end-to-end on this stack despite fewer
billed PE columns (and full-fp8 est+vp fails the 2e-2 gate at 2.7e-2;
s=6-only passes at 1.4e-2 but costs +106us); walrus --enable-ldw-opt
rejects this kernel's LDWEIGHTS mix; interleaving score strips into the
A@V sweep fragments the PE stream and loses ~20us to p-state/HAM;
6 per-piece AllReduces serialize on ~10-25us/collective rendezvous.
"""
import sys
import types

import numpy as np

F, P, H, HEADS, C, NCLS = 18, 128, 256, 8, 32, 625
SCALES = (2, 4, 6)
HD = H // HEADS
NTOK = F * P  # 2304
NCORES = 8

# frame block pitch inside vp (ones+V layout with zero guards)
_VPITCH = 352  # 16-aligned V blocks: ones at 79+112*si, V at 80+112*si


def _enable_ldw_opt():
    """Walrus's LDWEIGHTS dedup is disabled by default in bass_utils;
    enable it (verified numerically by the rel-err gate in test.py)."""
    import concourse.bass_utils as bu

    if getattr(bu, "_ldw_opt_patched", False):
        return
    orig = bu.bir_verify_and_optimise

    def patched(*args, **kwargs):
        real_run = bu.run_command

        def run_hook(argv, **kw):
            argv = ["--enable-ldw-opt=true" if a == "--enable-ldw-opt=false"
                    else a for a in argv]
            return real_run(argv, **kw)

        bu.run_command = run_hook
        try:
            return orig(*args, **kwargs)
        finally:
            bu.run_command = real_run

    bu.bir_verify_and_optimise = patched
    bu._ldw_opt_patched = True


def _install_ntff_hook():
    """Recreate the missing antenv.axon_hooks so trace=True works."""
    if "antenv.axon_hooks" in sys.modules:
        return
    try:
        import antenv

        mod = types.ModuleType("antenv.axon_hooks")
        mod._hook = None
        mod.set_axon_ntff_profile_hook = lambda h: setattr(mod, "_hook", h)
        mod.get_axon_ntff_profile_hook = lambda: mod._hook
        sys.modules["antenv.axon_hooks"] = mod
        antenv.axon_hooks = mod
        from trn_agent_boot.trn_boot import _ntff_profile_via_ctypes

        mod.set_axon_ntff_profile_hook(
            _ntff_profile_via_ctypes("/opt/axon/libaxon_pjrt.so")
        )
    except Exception:
        pass


def _chunks(n, lim=512):
    out = [lim] * (n // lim)
    if n % lim:
        out.append(n % lim)
    return out


def _counts(s):
    nw = F - s + 1
    c = np.zeros(F, np.float32)
    for w in range(nw):
        c[w:w + s] += 1.0
    return c


def _strip_meta(s):
    """Per key-frame strip [a, b] ranges and col offsets in the est tile."""
    offs, rng = [], []
    off = 0
    for f2 in range(F):
        a = max(0, f2 - s + 1)
        b = min(F - 1, f2 + s - 1)
        offs.append(off)
        rng.append((a, b))
        off += (b - a + 1) * 128
    return offs, rng, off


def _est_slack(s, meta):
    """Extra est cols needed so the [p, 2, D] pair-view stays in bounds."""
    offs, rng, tot = meta
    nw = F - s + 1
    slack = 0

    def qoff(w, j):
        return offs[j] + (w - rng[j][0]) * 128

    for w in range(nw):
        for t in range(s // 2):
            j0 = w + 2 * t
            base = qoff(w, j0)
            D = qoff(w, j0 + 1) - base
            off = 0
            for ck in _chunks(s * 128):
                assert D >= ck, (s, w, t, D, ck)
                slack = max(slack, base + off + 2 * D - tot)
                off += ck
    return slack


def build():
    import concourse.bacc as bacc
    import concourse.mybir as mybir
    from concourse.tile import TileContext

    dt = mybir.dt
    f32 = dt.float32
    bf16 = dt.bfloat16
    f8 = dt.float8e4
    AF = mybir.ActivationFunctionType
    ALU = mybir.AluOpType
    DR = mybir.MatmulPerfMode.DoubleRow

    nc = bacc.Bacc("TRN2", target_bir_lowering=False, debug=False,
                   num_devices=NCORES)

    # ---- I/O ----
    xt_in = nc.dram_tensor("xt", [2, 128, NTOK], bf16, kind="ExternalInput")
    wq_in = nc.dram_tensor("wq", [2, 128, 96], bf16, kind="ExternalInput")
    wk_in = nc.dram_tensor("wk", [2, 128, 96], bf16, kind="ExternalInput")
    wv_in = nc.dram_tensor("wv", [2, 128, 97], bf16, kind="ExternalInput")
    # row 96 of wo carries bo.sum (bias folded into the matmul via the
    # constant-ones column of abar / asum)
    wo_in = nc.dram_tensor("wo", [97, 256], bf16, kind="ExternalInput")
    nhb_in = nc.dram_tensor("nhb", [18, 1], f32, kind="ExternalInput")
    cinvT_in = nc.dram_tensor("cinvT", [96, F], f32, kind="ExternalInput")
    clsw_in = nc.dram_tensor("clsw", [2, 128, NCLS], f32, kind="ExternalInput")
    clsb_in = nc.dram_tensor("clsb", [1, NCLS], f32, kind="ExternalInput")
    id_in = nc.dram_tensor("ident", [128, 128], f32, kind="ExternalInput")
    out_d = nc.dram_tensor("out", [1, NCLS], f32, kind="ExternalOutput")

    # piece-major AllReduce bounce buffers: [piece, half, 128, 384]
    ar_in = nc.dram_tensor("ar_in", [6, 2, 128, 384], bf16)
    ar_out = nc.dram_tensor("ar_out", [6, 2, 128, 384], bf16,
                            addr_space="Shared")
    bar_in = nc.dram_tensor("bar_in", [1, 1], f32)
    bar_out = nc.dram_tensor("bar_out", [1, 1], f32, addr_space="Shared")
    ar2_in = nc.dram_tensor("ar2_in", [1, NCLS], f32)
    ar2_out = nc.dram_tensor("ar2_out", [1, NCLS], f32, addr_space="Shared")
    hbounce = nc.dram_tensor("hbounce", [18, 128], bf16)

    col_cc = _chunks(NTOK)  # [512]*4 + [256]
    meta = {s: _strip_meta(s) for s in SCALES}
    slack = {s: _est_slack(s, meta[s]) for s in SCALES}

    with TileContext(nc) as tc:
        with (
            tc.tile_pool(name="pin", bufs=1) as pin,
            tc.tile_pool(name="work", bufs=3) as work,
        ):
            # ---- persistent tiles + weight loads ----
            xt = [[pin.tile([128, 384], bf16, tag=f"xt{c}{p}",
                            name=f"xt{c}{p}") for p in range(6)]
                  for c in range(2)]
            wq_t = pin.tile([128, 2 * 96], bf16, tag="wq")
            wk_t = pin.tile([128, 2 * 96], bf16, tag="wk")
            wv_t = pin.tile([128, 2 * 97], bf16, tag="wv")
            wo_t = pin.tile([97, 256], bf16, tag="wo")
            nhb = pin.tile([18, 1], f32, tag="nhb")
            cinvT = pin.tile([96, F], f32, tag="cinvT")
            clsw = pin.tile([128, 2 * NCLS], f32, tag="clsw")
            clsb = pin.tile([1, NCLS], f32, tag="clsb")
            ident = pin.tile([128, 128], f32, tag="ident")
            identb = pin.tile([128, 128], bf16, tag="identb")
            ones_row = pin.tile([1, 128], f32, tag="ones_row")
            onesb = pin.tile([128, 1], bf16, tag="onesb")

            for c in range(2):
                for p in range(6):
                    eng = nc.sync if p % 2 == 0 else nc.gpsimd
                    eng.dma_start(out=xt[c][p][:],
                                  in_=xt_in[c, :, p * 384:(p + 1) * 384])
                nc.gpsimd.dma_start(out=wq_t[:, c * 96:(c + 1) * 96],
                                    in_=wq_in[c])
                nc.gpsimd.dma_start(out=wk_t[:, c * 96:(c + 1) * 96],
                                    in_=wk_in[c])
                nc.sync.dma_start(out=wv_t[:, c * 97:(c + 1) * 97], in_=wv_in[c])
                nc.gpsimd.dma_start(out=clsw[:, c * NCLS:(c + 1) * NCLS],
                                    in_=clsw_in[c])
            nc.gpsimd.dma_start(out=wo_t[:], in_=wo_in[:])
            nc.sync.dma_start(out=nhb[:], in_=nhb_in[:])
            nc.sync.dma_start(out=cinvT[:], in_=cinvT_in[:])
            nc.gpsimd.dma_start(out=clsb[:], in_=clsb_in[:])
            nc.gpsimd.dma_start(out=ident[:], in_=id_in[:])
            nc.vector.memset(ones_row[:], 1.0)
            nc.vector.tensor_copy(identb[:], ident[:])
            with nc.allow_low_precision(reason="bf16 ones, exact"):
                nc.vector.memset(onesb[:], 1.0)

            # grouped projections (token cols)
            QT = pin.tile([96, NTOK], bf16, tag="QT")
            KT = pin.tile([96, NTOK], bf16, tag="KT")
            VT = pin.tile([97, NTOK], bf16, tag="VT")
            # V' tile: per frame [zeros | 1 | V(3 scales) | zeros]
            vp = pin.tile([128, F * _VPITCH + 64], bf16, tag="vp")
            nc.vector.memset(vp[:], 0.0)
            for si in range(3):
                nc.vector.memset(
                    vp[:, :F * _VPITCH].rearrange(
                        "p (f c) -> p f c", c=_VPITCH)
                    [:, :, 79 + 112 * si:80 + 112 * si], 1.0)

            # est strips per scale
            est = {s: pin.tile([128, meta[s][2]], bf16,
                               tag=f"est{s}", name=f"est{s}")
                   for s in SCALES}
            # token-major attention accum, frame pitch 128 ([s2|s4|s6|junk])
            abar = pin.tile([128, F * 128], bf16, tag="abar")
            abarT = [pin.tile([128, 384], bf16, tag=f"abarT{p}",
                              name=f"abarT{p}") for p in range(6)]

            # halting state
            ptn = pin.tile([1, F], f32, tag="ptn")
            Rt = pin.tile([1, F], f32, tag="Rt")
            wts = [pin.tile([1, F], f32, tag=f"w{it}", name=f"w{it}")
                   for it in range(2)]
            ssum = [[pin.tile([128, F], f32, tag=f"ssum{it}{c}",
                              name=f"ssum{it}{c}") for c in range(2)]
                    for it in range(2)]
            halt18 = pin.tile([18, 128], bf16, tag="halt18")
            asum_t = pin.tile([97, F], bf16, tag="asum")
            nc.vector.memset(ptn[:], 0.0)
            nc.vector.memset(Rt[:], 0.0)

            def emit_qkv(it, pieces, tagx):
                with tc.tile_pool(name=f"pq{it}{tagx}",
                                  bufs=len(pieces) + 1, space="PSUM") as ppq:
                    for gi, (wt, gt, rows) in enumerate(
                            ((wv_t, VT, 97), (wq_t, QT, 96), (wk_t, KT, 96))):
                        ptile = {}
                        for hc in range(2):
                            for p in pieces:
                                if hc == 0:
                                    ptile[p] = ppq.tile(
                                        [97, 384], f32, tag="pg", name="pg")
                                nc.tensor.matmul(
                                    ptile[p][:rows, :],
                                    wt[:, hc * rows:(hc + 1) * rows],
                                    xt[hc][p][:],
                                    start=(hc == 0), stop=(hc == 1))
                                if hc == 1:
                                    gc = p * 384
                                    if p % 2 == 0:
                                        nc.scalar.copy(
                                            gt[:, gc:gc + 384],
                                            ptile[p][:rows, :])
                                    else:
                                        nc.vector.tensor_copy(
                                            gt[:, gc:gc + 384],
                                            ptile[p][:rows, :])

            def emit_vprime(it, frames, tagx):
                with tc.tile_pool(name=f"pv{it}{tagx}", bufs=2,
                                  space="PSUM") as ppv:
                    for t in frames:
                        pvt = ppv.tile([128, 96], bf16, tag="pvt")
                        nc.tensor.transpose(
                            pvt[:], VT[0:96, t * 128:(t + 1) * 128],
                            identb[0:96, 0:96])
                        dst = vp[:, t * _VPITCH + 80:
                                 t * _VPITCH + 80 + 3 * 112]
                        nc.vector.tensor_copy(
                            dst.rearrange("p (s c) -> p s c", c=112)
                            [:, :, 0:32],
                            pvt[:].rearrange("p (s c) -> p s c", c=32))

            def emit_scores(it, fmax, tagx):
                # strips whose query range ends at frame <= fmax
                with tc.tile_pool(name=f"ps{it}{tagx}", bufs=2,
                                  space="PSUM") as pps:
                    for si, s in enumerate(SCALES):
                        offs, rng, _tot = meta[s]
                        for f2 in range(F):
                            a, b = rng[f2]
                            if not (b <= fmax if tagx == "a" else b > fmax):
                                continue
                            ncols = (b - a + 1) * 128
                            pstr = pps.tile([128, 11 * 128], f32, tag="pstr")
                            off = 0
                            for w_cc in _chunks(ncols):
                                nc.tensor.matmul(
                                    pstr[:, off:off + w_cc],
                                    KT[32 * si:32 * (si + 1),
                                       f2 * 128:(f2 + 1) * 128],
                                    QT[32 * si:32 * (si + 1),
                                       a * 128 + off:a * 128 + off + w_cc],
                                    start=True, stop=True)
                                off += w_cc
                            nc.scalar.activation(
                                est[s][:, offs[f2]:offs[f2] + ncols],
                                pstr[:, :ncols], AF.Exp)

            def emit_halting(it):
                # moved ahead of the A@V phase: only depends on halt18, and
                # its weights gate nothing until the final combine.
                with tc.tile_pool(name=f"ph{it}", bufs=1, space="PSUM") as pph:
                    elog = work.tile([18, 128], f32, tag="elog")
                    nc.scalar.activation(elog[:], halt18[:],
                                         AF.Exp, bias=nhb[:], scale=-1.0)
                    nc.vector.tensor_scalar_add(out=elog[:], in0=elog[:],
                                                scalar1=1.0)
                    sig = work.tile([18, 128], f32, tag="sig")
                    nc.vector.reciprocal(sig[:], elog[:])
                    pred = work.tile([18, 1], f32, tag="pred")
                    nc.vector.tensor_reduce(out=pred[:], in_=sig[:],
                                            axis=mybir.AxisListType.X,
                                            op=ALU.add)
                    ptp = pph.tile([1, F], f32, tag="pt")
                    nc.tensor.transpose(ptp[:], pred[:], ident[0:18, 0:18])
                    p_t = work.tile([1, F], f32, tag="p_t")
                    nc.vector.tensor_scalar_mul(out=p_t[:], in0=ptp[:],
                                                scalar1=1.0 / 128.0)

                    # halting state updates (elementwise on [1,F])
                    run_in = work.tile([1, F], f32, tag="run_in")
                    tmp = work.tile([1, F], f32, tag="tmp")
                    tmp2 = work.tile([1, F], f32, tag="tmp2")
                    nh = work.tile([1, F], f32, tag="nh")
                    run = work.tile([1, F], f32, tag="run")
                    nc.vector.tensor_scalar(out=run_in[:], in0=ptn[:],
                                            scalar1=1.0, scalar2=None,
                                            op0=ALU.is_lt)
                    nc.vector.tensor_tensor(out=tmp[:], in0=p_t[:],
                                            in1=run_in[:], op=ALU.mult)
                    nc.vector.tensor_tensor(out=tmp2[:], in0=ptn[:],
                                            in1=tmp[:], op=ALU.add)
                    nc.vector.tensor_scalar(out=tmp2[:], in0=tmp2[:],
                                            scalar1=0.99, scalar2=None,
                                            op0=ALU.is_gt)
                    nc.vector.tensor_tensor(out=nh[:], in0=tmp2[:],
                                            in1=run_in[:], op=ALU.mult)
                    nc.vector.tensor_tensor(out=run[:], in0=run_in[:],
                                            in1=nh[:], op=ALU.subtract)
                    nc.vector.tensor_tensor(out=tmp[:], in0=p_t[:],
                                            in1=run[:], op=ALU.mult)
                    nc.vector.tensor_tensor(out=ptn[:], in0=ptn[:],
                                            in1=tmp[:], op=ALU.add)
                    nc.vector.tensor_scalar(out=tmp2[:], in0=ptn[:],
                                            scalar1=-1.0, scalar2=1.0,
                                            op0=ALU.mult, op1=ALU.add)
                    nc.vector.tensor_tensor(out=tmp2[:], in0=nh[:],
                                            in1=tmp2[:], op=ALU.mult)
                    nc.vector.tensor_tensor(out=Rt[:], in0=Rt[:],
                                            in1=tmp2[:], op=ALU.add)
                    nc.vector.tensor_tensor(out=tmp2[:], in0=nh[:],
                                            in1=Rt[:], op=ALU.mult)
                    nc.vector.tensor_tensor(out=ptn[:], in0=ptn[:],
                                            in1=tmp2[:], op=ALU.add)
                    nc.vector.tensor_tensor(out=wts[it][:], in0=tmp[:],
                                            in1=tmp2[:], op=ALU.add)
                    if it == 1:
                        # context-mix coefficients; overlaps iter-2 scores
                        w1, w2 = wts[0], wts[1]
                        c1 = work.tile([1, 2 * F], f32, tag="coef")
                        nc.vector.tensor_copy(c1[:, 0:F], w2[:])
                        tmpc = work.tile([1, F], f32, tag="tmpc")
                        nc.vector.tensor_scalar(out=tmpc[:], in0=w2[:],
                                                scalar1=-1.0, scalar2=1.0,
                                                op0=ALU.mult, op1=ALU.add)
                        nc.vector.tensor_tensor(out=tmpc[:], in0=tmpc[:],
                                                in1=w1[:], op=ALU.mult)
                        nc.vector.tensor_scalar_mul(out=c1[:, F:2 * F],
                                                    in0=tmpc[:],
                                                    scalar1=1.0 / NCORES)
                        pcc = pph.tile([128, 2 * F], f32, tag="pc")
                        nc.tensor.matmul(pcc[:], ones_row[:], c1[:],
                                         start=True, stop=True)
                        nc.vector.tensor_copy(coefb[:], pcc[:])

            coefb = pin.tile([128, 2 * F], f32, tag="coefb")

            for it in range(2):
                def cb_chunk(plo, phi):
                    # AllReduce copy-backs on the ACT queue: its next work
                    # (the QKV drains) waits on this data anyway, so no
                    # engine loses issue slots behind the collective.
                    for p in range(plo, phi):
                        for hc in range(2):
                            nc.scalar.dma_start(out=xt[hc][p][:],
                                                in_=ar_out[p, hc])
                    for p in range(plo, phi):
                        for hc in range(2):
                            nc.vector.tensor_reduce(
                                out=ssum[0][hc][:, 3 * p:3 * p + 3],
                                in_=xt[hc][p][:].rearrange(
                                    "q (f c) -> q f c", c=128),
                                axis=mybir.AxisListType.X, op=ALU.add)

                if it == 0:
                    emit_qkv(0, range(6), "a")
                else:
                    # iter-2 head split by AR chunk: pieces 0-2 (chunk 1)
                    # first, so their QKV/V'/scores hide the second AllReduce
                    # instead of head-blocking the PE queue behind it.
                    cb_chunk(0, 3)
                    emit_qkv(1, range(3), "a")
                    emit_vprime(1, range(9), "a")
                    emit_scores(1, 8, "a")
                    cb_chunk(3, 6)
                    emit_qkv(1, range(3, 6), "b")

                nc.sync.dma_start(out=hbounce[:].rearrange("f p -> (f p)"),
                                  in_=VT[96:97, :])
                nc.sync.dma_start(out=halt18[:], in_=hbounce[:])

                if it == 0:
                    emit_vprime(0, range(F), "a")
                else:
                    emit_vprime(1, range(9, F), "b")

                # token-major accum cleared per iteration; col 96 of
                # each frame block is a constant 1 that becomes abarT row
                # 96 / asum row 96, turning the Wo bias into a matmul row.
                with nc.allow_low_precision(reason="bf16 abar, 2e-2 gate"):
                    nc.gpsimd.memset(abar[:], 0.0)
                    nc.gpsimd.memset(
                        abar[:].rearrange("p (f c) -> p f c", c=128)
                        [:, :, 96:97], 1.0)

                emit_scores(it, 8 if it == 1 else -1, "b")
                emit_halting(it)

                if it == 0:
                    # skew-absorbing barriers, hidden under attention
                    nc.sync.dma_start(out=bar_in[:], in_=nhb[0:1, 0:1])
                    for _ in range(2):
                        nc.gpsimd.collective_compute(
                            "AllReduce", ALU.add,
                            ins=[bar_in[:]], outs=[bar_out[:]],
                            replica_groups=[list(range(NCORES))])

                # ============ A@V + overlap-add ============
                with (
                    tc.tile_pool(name=f"pa{it}", bufs=2, space="PSUM") as ppa,
                    tc.tile_pool(name=f"pt{it}", bufs=2, space="PSUM") as ppt,
                    tc.tile_pool(name=f"pw{it}", bufs=2, space="PSUM") as ppw,
                ):
                    if it == 1:
                        asum_ps = ppw.tile([97, F], f32, tag="asum_ps",
                                           name="asum_ps", bufs=1)

                    def drain_group(si, s, g, gw, gtile):
                        sP = s * 128
                        av = work.tile([99, 768], bf16, tag="av_sb",
                                       bufs=3, name="av_sb")
                        # s=2/s=4 drains overlap the exp phase (ACT busy)
                        # -> vector; s=6 drains run after exp is done ->
                        # scalar, freeing vector for the rescale tail.
                        if si == 2:
                            nc.scalar.copy(av[:, :sP], gtile[:, :sP])
                        else:
                            nc.vector.tensor_copy(av[:, :sP], gtile[:, :sP])
                        ptg = ppt.tile([128, 6 * 132], bf16, tag="ptT")
                        for qc in range(s):
                            nc.tensor.transpose(
                                ptg[:, qc * 132:qc * 132 + 99],
                                av[:, qc * 128:(qc + 1) * 128],
                                identb[0:99, 0:99])
                        ptv = ptg[:, :s * 132].rearrange(
                            "p (q w v) -> p q w v", w=4, v=33)
                        rcp = work.tile([128, 18], f32, tag="rcp")
                        rcpv = rcp[:, :s * gw].rearrange(
                            "p (q w) -> p q w", w=gw)
                        nc.vector.reciprocal(rcpv, ptv[:, :, :gw, 0])
                        ab = abar[:].rearrange("p (f c) -> p f c", c=128)
                        for qc in range(s):
                            resc = work.tile([128, 96], f32, tag="resc")
                            rv = resc[:].rearrange("p (w c) -> p w c", c=32)
                            nc.vector.tensor_tensor(
                                out=rv[:, :gw, :], in0=ptv[:, qc, :gw, 1:33],
                                in1=rcpv[:, qc, :gw].broadcast_to(
                                    (128, gw, 32)),
                                op=ALU.mult)
                            with nc.allow_low_precision(
                                    reason="bf16 abar, 2e-2 gate"):
                                # accumulate on gpsimd (SBUF-only op): it
                                # unclogs the vector queue so the piece
                                # chains reach the collectives sooner.
                                nc.gpsimd.tensor_tensor(
                                    out=ab[:, 3 * g + qc:3 * g + qc + gw,
                                           si * 32:(si + 1) * 32],
                                    in0=ab[:, 3 * g + qc:3 * g + qc + gw,
                                           si * 32:(si + 1) * 32],
                                    in1=rv[:, :gw, :], op=ALU.add)

                    def frame_done(f):
                        # alternate the two hardware DMA queues (SP/ACT):
                        # serialized on one ring these 18 transposes cost
                        # ~22us on the piece->AllReduce critical chain.
                        p, fo = f // 3, f % 3
                        eng = nc.sync if f % 2 == 0 else nc.scalar
                        eng.dma_start_transpose(
                            out=abarT[p][:, fo * 128:(fo + 1) * 128],
                            in_=abar[:].rearrange("p (f c) -> p f c", c=128)
                            [:, f, :])

                    def piece_done(p):
                        # fold 1/overlap-counts in, project through Wo,
                        # stage for the chunked AllReduce.  The elementwise
                        # steps run on gpsimd/scalar so the busy vector
                        # queue never delays the collective's inputs.
                        abt = abarT[p][0:96, :].rearrange(
                            "q (f c) -> q f c", c=128)
                        cv = cinvT[:, 3 * p:3 * p + 3].broadcast_to(
                            (96, 3, 128))
                        with nc.allow_low_precision(
                                reason="bf16 abarT, 2e-2 gate"):
                            nc.gpsimd.tensor_tensor(out=abt, in0=abt, in1=cv,
                                                    op=ALU.mult)
                        for hc in range(2):
                            pw = ppw.tile([128, 384], f32, tag="pw")
                            nc.tensor.matmul(
                                pw[:], wo_t[:, hc * 128:(hc + 1) * 128],
                                abarT[p][0:97, :], start=True, stop=True)
                            with nc.allow_low_precision(
                                    reason="bf16 xt, 2e-2 gate"):
                                nc.scalar.activation(
                                    xt[hc][p][:], pw[:], AF.Copy,
                                    scale=0.25)
                            nc.sync.dma_start(out=ar_in[p, hc],
                                              in_=xt[hc][p][:])

                    def ar_chunk(plo, phi):
                        # trigger only; the copy-backs are issued from the
                        # ACT queue in the iter-2 head, right before the QKV
                        # drains that need the data, so no other queue ever
                        # stalls behind the collective.
                        nc.gpsimd.collective_compute(
                            "AllReduce", ALU.add,
                            ins=[ar_in[plo:phi]], outs=[ar_out[plo:phi]],
                            replica_groups=[list(range(NCORES))])

                    for si, s in enumerate(SCALES):
                        nw = F - s + 1
                        sP = s * 128
                        offs, rng, _tot = meta[s]
                        ngrp = (nw + 2) // 3

                        def qoff(w, j, offs=offs, rng=rng):
                            return offs[j] + (w - rng[j][0]) * 128

                        for g in range(ngrp):
                            gw = min(3, nw - 3 * g)
                            gtile = ppa.tile([99, 768], f32, tag="grp",
                                             name="grp")
                            for wi in range(gw):
                                w = 3 * g + wi
                                for j in range(w, w + s):
                                    base = qoff(w, j)
                                    st = (wi == 0 and j == w)
                                    sp = (wi == gw - 1 and j == w + s - 1)
                                    lo = (j * _VPITCH + 79 +
                                          112 * si - 33 * wi)
                                    lhs = vp[:, lo:lo + 99]
                                    off = 0
                                    for w_cc in _chunks(sP):
                                        nc.tensor.matmul(
                                            gtile[0:99, off:off + w_cc],
                                            lhs,
                                            est[s][:, base + off:
                                                   base + off + w_cc],
                                            start=st, stop=sp)
                                        off += w_cc
                            drain_group(si, s, g, gw, gtile)
                            if si == 2:
                                lof = 3 * g
                                hif = 3 * g + 3 if g < ngrp - 1 else F
                                if it == 0:
                                    for f in range(lof, hif):
                                        frame_done(f)
                                    # piece_done delayed one group so its
                                    # PE matmuls never head-block the next
                                    # group's A@V work while waiting on the
                                    # drain/transpose chain.
                                    if g > 0:
                                        piece_done(g - 1)
                                    if g == 3:
                                        ar_chunk(0, 3)
                                    if g == ngrp - 1:
                                        piece_done(g)
                                        piece_done(5)
                                        ar_chunk(3, 6)
                                else:
                                    if g > 0:
                                        for f in range(3 * g - 3, 3 * g):
                                            nc.tensor.matmul(
                                                asum_ps[0:97, f:f + 1],
                                                abar[:, f * 128:
                                                     f * 128 + 97],
                                                onesb[:], start=True,
                                                stop=True)
                                    if g == ngrp - 1:
                                        for f in range(3 * g - 3 + 3, F):
                                            nc.tensor.matmul(
                                                asum_ps[0:97, f:f + 1],
                                                abar[:, f * 128:
                                                     f * 128 + 97],
                                                onesb[:], start=True,
                                                stop=True)

                    if it == 1:
                        asum = asum_t
                        with nc.allow_low_precision(
                                reason="bf16 frame sums, 2e-2 gate"):
                            nc.vector.tensor_copy(asum[:], asum_ps[:])
                            nc.vector.tensor_tensor(out=asum[0:96, :],
                                                    in0=asum[0:96, :],
                                                    in1=cinvT[:],
                                                    op=ALU.mult)
                        for hc in range(2):
                            ps2 = ppw.tile([128, F], f32, tag="ps2",
                                           name="ps2", bufs=1)
                            nc.tensor.matmul(
                                ps2[:], wo_t[:, hc * 128:(hc + 1) * 128],
                                asum[:], start=True, stop=True)
                            nc.scalar.activation(
                                ssum[1][hc][:], ps2[:], AF.Copy,
                                scale=0.25)

            # ============ final combine (per-core, then output AR) ========
            with tc.tile_pool(name="pf", bufs=1, space="PSUM") as ppf:
                vfull = [work.tile([128, 1], f32, tag=f"vfull{hc}",
                                   name=f"vfull{hc}") for hc in range(2)]
                ob = work.tile([1, NCLS], f32, tag="ob")
                for hc in range(2):
                    t2 = work.tile([128, F], f32, tag="t2")
                    nc.vector.tensor_tensor(out=t2[:], in0=ssum[1][hc][:],
                                            in1=coefb[:, 0:F], op=ALU.mult)
                    t1 = work.tile([128, F], f32, tag="t1")
                    nc.vector.tensor_tensor(out=t1[:], in0=ssum[0][hc][:],
                                            in1=coefb[:, F:2 * F], op=ALU.mult)
                    nc.vector.tensor_tensor(out=t2[:], in0=t2[:], in1=t1[:],
                                            op=ALU.add)
                    nc.vector.tensor_reduce(out=vfull[hc][:], in_=t2[:],
                                            axis=mybir.AxisListType.X,
                                            op=ALU.add)
                off = 0
                for w_cc in _chunks(NCLS):
                    pcls = ppf.tile([1, 512], f32, tag="pcls")
                    for hc in range(2):
                        nc.tensor.matmul(
                            pcls[:, :w_cc], vfull[hc][:],
                            clsw[:, hc * NCLS + off:hc * NCLS + off + w_cc],
                            start=(hc == 0), stop=(hc == 1))
                    # clsb is pre-scaled by 1/NCORES host-side; the output
                    # AllReduce restores it.
                    nc.vector.tensor_tensor(out=ob[:, off:off + w_cc],
                                            in0=pcls[:, :w_cc],
                                            in1=clsb[:, off:off + w_cc],
                                            op=ALU.add)
                    off += w_cc
                nc.sync.dma_start(out=ar2_in[:], in_=ob[:])
                nc.gpsimd.collective_compute(
                    "AllReduce", ALU.add,
                    ins=[ar2_in[:]], outs=[ar2_out[:]],
                    replica_groups=[list(range(NCORES))])
                ob2 = work.tile([1, NCLS], f32, tag="ob2")
                nc.sync.dma_start(out=ob2[:], in_=ar2_out[:])
                nc.sync.dma_start(out=out_d[:], in_=ob2[:])

    nc.compile()
    return nc


_NC_CACHE = None


def _get_nc():
    global _NC_CACHE
    if _NC_CACHE is None:
        _NC_CACHE = build()
    return _NC_CACHE


def _prep_in_maps(inputs):
    emb = np.ascontiguousarray(np.asarray(inputs["multiscale_embed"], np.float32))
    halt_W = np.asarray(inputs["halt_W"], np.float32)
    halt_b = np.asarray(inputs["halt_b"], np.float32)
    cls_W = np.asarray(inputs["cls_W"], np.float32)
    cls_b = np.asarray(inputs["cls_b"], np.float32)
    Wq = np.asarray(inputs["mhsa_Wq"], np.float32)
    Wk = np.asarray(inputs["mhsa_Wk"], np.float32)
    Wv = np.asarray(inputs["mhsa_Wv"], np.float32)
    Wo = np.asarray(inputs["mhsa_Wo"], np.float32)
    bo = np.asarray(inputs["mhsa_bo"], np.float32)

    import ml_dtypes
    bf = ml_dtypes.bfloat16
    xt = np.ascontiguousarray(
        emb.reshape(NTOK, H).T.reshape(2, 128, NTOK)).astype(bf)
    bosum = bo.sum(axis=0)[None, :]  # [1, 256]; Wo row 96
    hwc = halt_W.reshape(256)
    nhb = np.full((18, 1), -float(halt_b[0]), np.float32)
    cinvT = np.concatenate(
        [np.repeat((1.0 / _counts(s))[None, :], 32, axis=0) for s in SCALES]
    ).astype(np.float32)  # [96, F]
    clsw = np.ascontiguousarray(cls_W.reshape(2, 128, NCLS))
    # the final combine adds clsb per-core before the output AllReduce
    clsb = (cls_b.reshape(1, NCLS) / NCORES).astype(np.float32)
    ident = np.eye(128, dtype=np.float32)
    inv_sqrt_hd = 1.0 / np.sqrt(np.float32(HD))

    in_maps = []
    for h in range(NCORES):
        sl = slice(h * HD, (h + 1) * HD)
        # grouped weights: cols = [scale2 | scale4 | scale6] head-slices
        wq_g = np.concatenate([Wq[si][:, sl] for si in range(3)], axis=1)
        wk_g = np.concatenate([Wk[si][:, sl] * inv_sqrt_hd for si in range(3)],
                              axis=1)
        wv_g = np.concatenate([Wv[si][:, sl] for si in range(3)] +
                              [hwc[:, None]], axis=1)  # [256, 97]
        wo_g = np.concatenate([Wo[si][sl, :] for si in range(3)] +
                              [bosum], axis=0)  # [97, 256]
        in_maps.append({
            "xt": xt,
            "wq": np.ascontiguousarray(wq_g.reshape(2, 128, 96)).astype(bf),
            "wk": np.ascontiguousarray(wk_g.reshape(2, 128, 96)).astype(bf),
            "wv": np.ascontiguousarray(wv_g.reshape(2, 128, 97)).astype(bf),
            "wo": np.ascontiguousarray(wo_g).astype(bf),
            "nhb": nhb, "cinvT": cinvT, "clsw": clsw,
            "clsb": clsb, "ident": ident,
        })
    return in_maps


def run(inputs, trace=False):
    _install_ntff_hook()
    # NOTE: _enable_ldw_opt() stays off: walrus's LDW dedup rejects this
    # kernel's LDWEIGHTS mix ("InstLdweights is not compatible with LDW
    # optimization"), with or without DoubleRow.
    from concourse.bass_utils import run_bass_kernel_spmd

    nc = _get_nc()
    in_maps = _prep_in_maps(inputs)
    res = run_bass_kernel_spmd(nc, in_maps, list(range(NCORES)), trace=trace)
    out = np.asarray(res.results[0]["out"], np.float32)
    return out, res


def kernel(**inputs):
    out, _ = run(inputs, trace=False)
    return out


# revision 49
# speedup vs baseline: 1.1015x; 1.1015x over previous
"""Trainium2 Bass kernel for nn_CRF_SelfAttention_65627100283470.

Math (validated vs the reference at 1e-6 rel err):
  - The CRF/marginal branch is dead code: softmax over the class dim sums
    to 1, so sum(cluster_features, 0) == sum of context rows.  The output
    is (sum_{f,p} context2) @ cls_W + cls_b.
  - context2 = w2*T2 + w1*(1-w2)*T1 with T_it the per-iteration temporal
    tensors, and w_it per-frame halting weights -> only per-frame sums of
    temporal are needed at the end.
  - QKV projections are shared across overlapping windows; exp(scores)
    strips are shared across windows (computed per key-frame strip); the
    output projection commutes with overlap-add; softmax denominators come
    from a ones-column prepended to V.
  - The 1/overlap-count scaling commutes with the A@V drain (it is a
    per-(scale, query-frame) scalar), so it is applied once per piece to
    abarT (iter 1) / once to asum (iter 2) instead of per window.

Sharding: 8 heads -> 8 cores.  The mid-kernel temporal exchange is TWO
chunked AllReduces (pieces 0-2 fired mid-s=6-sweep, pieces 3-5 at the
end) whose copy-backs issue from the ACT queue right before the iter-2
QKV that consumes them; iter-2's head is emitted in two piece-halves so
chunk-1 work hides chunk-2's collective.  The final collective reduces
the [1,625] class output itself (coefs/ssum0 are replicated).

This revision (vs the 383us v1), all bf16 on the PE:
  - abar is bf16 with a constant-ones column per frame; Wo is a 97-row
    matrix whose last row carries bo.sum, folding the bias into the
    matmul so both Wo drains are single scaled copies on ACT.
  - Wo projection + AllReduce staging run per 3-frame piece, pipelined
    one group behind the s=6 A@V sweep (PE never head-blocks on the
    drain/transpose chain).
  - iter-2 frame sums via tiny PE column-sum matmuls (no DMA transposes)
    and the halting/coef chains hoisted to the iteration head.
  - drain: one batched reciprocal per group (132-pitch transposed
    tiles); overlap-add accumulation on gpsimd; overlap-count scaling
    applied once per piece (abarT) / once to asum instead of per window.

Measured dead ends (do not revisit without new evidence): fp8e4
DoubleRow A@V is ~30% SLOWER end-to-end on this stack despite fewer
billed PE columns (and full-fp8 est+vp fails the 2e-2 gate at 2.7e-2;
s=6-only passes at 1.4e-2 but costs +106us); walrus --enable-ldw-opt
rejects this kernel's LDWEIGHTS mix; interleaving score strips into the
A@V sweep fragments the PE stream and loses ~20us to p-state/HAM;
6 per-piece AllReduces serialize on ~10-25us/collective rendezvous.
"""
import sys
import types

import numpy as np

F, P, H, HEADS, C, NCLS = 18, 128, 256, 8, 32, 625
SCALES = (2, 4, 6)
HD = H // HEADS
NTOK = F * P  # 2304
NCORES = 8

# frame block pitch inside vp (ones+V layout with zero guards)
_VPITCH = 352  # 16-aligned V blocks: ones at 79+112*si, V at 80+112*si


def _enable_ldw_opt():
    """Walrus's LDWEIGHTS dedup is disabled by default in bass_utils;
    enable it (verified numerically by the rel-err gate in test.py)."""
    import concourse.bass_utils as bu

    if getattr(bu, "_ldw_opt_patched", False):
        return
    orig = bu.bir_verify_and_optimise

    def patched(*args, **kwargs):
        real_run = bu.run_command

        def run_hook(argv, **kw):
            argv = ["--enable-ldw-opt=true" if a == "--enable-ldw-opt=false"
                    else a for a in argv]
            return real_run(argv, **kw)

        bu.run_command = run_hook
        try:
            return orig(*args, **kwargs)
        finally:
            bu.run_command = real_run

    bu.bir_verify_and_optimise = patched
    bu._ldw_opt_patched = True


def _install_ntff_hook():
    """Recreate the missing antenv.axon_hooks so trace=True works."""
    if "antenv.axon_hooks" in sys.modules:
        return
    try:
        import antenv

        mod = types.ModuleType("antenv.axon_hooks")
        mod._hook = None
        mod.set_axon_ntff_profile_hook = lambda h: setattr(mod, "_hook", h)
        mod.get_axon_ntff_profile_hook = lambda: mod._hook
        sys.modules["antenv.axon_hooks"] = mod
        antenv.axon_hooks = mod
        from trn_agent_boot.trn_boot import _ntff_profile_via_ctypes

        mod.set_axon_ntff_profile_hook(
            _ntff_profile_via_ctypes("/opt/axon/libaxon_pjrt.so")
        )
    except Exception:
        pass


def _chunks(n, lim=512):
    out = [lim] * (n // lim)
    if n % lim:
        out.append(n % lim)
    return out


def _counts(s):
    nw = F - s + 1
    c = np.zeros(F, np.float32)
    for w in range(nw):
        c[w:w + s] += 1.0
    return c


def _strip_meta(s):
    """Per key-frame strip [a, b] ranges and col offsets in the est tile."""
    offs, rng = [], []
    off = 0
    for f2 in range(F):
        a = max(0, f2 - s + 1)
        b = min(F - 1, f2 + s - 1)
        offs.append(off)
        rng.append((a, b))
        off += (b - a + 1) * 128
    return offs, rng, off


def _est_slack(s, meta):
    """Extra est cols needed so the [p, 2, D] pair-view stays in bounds."""
    offs, rng, tot = meta
    nw = F - s + 1
    slack = 0

    def qoff(w, j):
        return offs[j] + (w - rng[j][0]) * 128

    for w in range(nw):
        for t in range(s // 2):
            j0 = w + 2 * t
            base = qoff(w, j0)
            D = qoff(w, j0 + 1) - base
            off = 0
            for ck in _chunks(s * 128):
                assert D >= ck, (s, w, t, D, ck)
                slack = max(slack, base + off + 2 * D - tot)
                off += ck
    return slack


def build():
    import concourse.bacc as bacc
    import concourse.mybir as mybir
    from concourse.tile import TileContext

    dt = mybir.dt
    f32 = dt.float32
    bf16 = dt.bfloat16
    f8 = dt.float8e4
    AF = mybir.ActivationFunctionType
    ALU = mybir.AluOpType
    DR = mybir.MatmulPerfMode.DoubleRow

    nc = bacc.Bacc("TRN2", target_bir_lowering=False, debug=False,
                   num_devices=NCORES)

    # ---- I/O ----
    xt_in = nc.dram_tensor("xt", [2, 128, NTOK], bf16, kind="ExternalInput")
    wq_in = nc.dram_tensor("wq", [2, 128, 96], bf16, kind="ExternalInput")
    wk_in = nc.dram_tensor("wk", [2, 128, 96], bf16, kind="ExternalInput")
    wv_in = nc.dram_tensor("wv", [2, 128, 97], bf16, kind="ExternalInput")
    # row 96 of wo carries bo.sum (bias folded into the matmul via the
    # constant-ones column of abar / asum)
    wo_in = nc.dram_tensor("wo", [97, 256], bf16, kind="ExternalInput")
    nhb_in = nc.dram_tensor("nhb", [18, 1], f32, kind="ExternalInput")
    cinvT_in = nc.dram_tensor("cinvT", [96, F], f32, kind="ExternalInput")
    clsw_in = nc.dram_tensor("clsw", [2, 128, NCLS], f32, kind="ExternalInput")
    clsb_in = nc.dram_tensor("clsb", [1, NCLS], f32, kind="ExternalInput")
    id_in = nc.dram_tensor("ident", [128, 128], f32, kind="ExternalInput")
    out_d = nc.dram_tensor("out", [1, NCLS], f32, kind="ExternalOutput")

    # piece-major AllReduce bounce buffers: [piece, half, 128, 384]
    ar_in = nc.dram_tensor("ar_in", [6, 2, 128, 384], bf16)
    ar_out = nc.dram_tensor("ar_out", [6, 2, 128, 384], bf16,
                            addr_space="Shared")
    bar_in = nc.dram_tensor("bar_in", [1, 1], f32)
    bar_out = nc.dram_tensor("bar_out", [1, 1], f32, addr_space="Shared")
    ar2_in = nc.dram_tensor("ar2_in", [1, NCLS], f32)
    ar2_out = nc.dram_tensor("ar2_out", [1, NCLS], f32, addr_space="Shared")
    hbounce = nc.dram_tensor("hbounce", [18, 128], bf16)

    col_cc = _chunks(NTOK)  # [512]*4 + [256]
    meta = {s: _strip_meta(s) for s in SCALES}
    slack = {s: _est_slack(s, meta[s]) for s in SCALES}

    with TileContext(nc) as tc:
        with (
            tc.tile_pool(name="pin", bufs=1) as pin,
            tc.tile_pool(name="work", bufs=3) as work,
        ):
            # ---- persistent tiles + weight loads ----
            xt = [[pin.tile([128, 384], bf16, tag=f"xt{c}{p}",
                            name=f"xt{c}{p}") for p in range(6)]
                  for c in range(2)]
            wq_t = pin.tile([128, 2 * 96], bf16, tag="wq")
            wk_t = pin.tile([128, 2 * 96], bf16, tag="wk")
            wv_t = pin.tile([128, 2 * 97], bf16, tag="wv")
            wo_t = pin.tile([97, 256], bf16, tag="wo")
            nhb = pin.tile([18, 1], f32, tag="nhb")
            cinvT = pin.tile([96, F], f32, tag="cinvT")
            clsw = pin.tile([128, 2 * NCLS], f32, tag="clsw")
            clsb = pin.tile([1, NCLS], f32, tag="clsb")
            ident = pin.tile([128, 128], f32, tag="ident")
            identb = pin.tile([128, 128], bf16, tag="identb")
            ones_row = pin.tile([1, 128], f32, tag="ones_row")
            onesb = pin.tile([128, 1], bf16, tag="onesb")

            for c in range(2):
                for p in range(6):
                    eng = nc.sync if p % 2 == 0 else nc.gpsimd
                    eng.dma_start(out=xt[c][p][:],
                                  in_=xt_in[c, :, p * 384:(p + 1) * 384])
                nc.gpsimd.dma_start(out=wq_t[:, c * 96:(c + 1) * 96],
                                    in_=wq_in[c])
                nc.gpsimd.dma_start(out=wk_t[:, c * 96:(c + 1) * 96],
                                    in_=wk_in[c])
                nc.sync.dma_start(out=wv_t[:, c * 97:(c + 1) * 97], in_=wv_in[c])
                nc.gpsimd.dma_start(out=clsw[:, c * NCLS:(c + 1) * NCLS],
                                    in_=clsw_in[c])
            nc.gpsimd.dma_start(out=wo_t[:], in_=wo_in[:])
            nc.sync.dma_start(out=nhb[:], in_=nhb_in[:])
            nc.sync.dma_start(out=cinvT[:], in_=cinvT_in[:])
            nc.gpsimd.dma_start(out=clsb[:], in_=clsb_in[:])
            nc.gpsimd.dma_start(out=ident[:], in_=id_in[:])
            nc.vector.memset(ones_row[:], 1.0)
            nc.vector.tensor_copy(identb[:], ident[:])
            with nc.allow_low_precision(reason="bf16 ones, exact"):
                nc.vector.memset(onesb[:], 1.0)

            # grouped projections (token cols)
            QT = pin.tile([96, NTOK], bf16, tag="QT")
            KT = pin.tile([96, NTOK], bf16, tag="KT")
            VT = pin.tile([97, NTOK], bf16, tag="VT")
            # V' tile: per frame [zeros | 1 | V(3 scales) | zeros]
            vp = pin.tile([128, F * _VPITCH + 64], bf16, tag="vp")
            nc.vector.memset(vp[:], 0.0)
            for si in range(3):
                nc.vector.memset(
                    vp[:, :F * _VPITCH].rearrange(
                        "p (f c) -> p f c", c=_VPITCH)
                    [:, :, 79 + 112 * si:80 + 112 * si], 1.0)

            # est strips per scale
            est = {s: pin.tile([128, meta[s][2]], bf16,
                               tag=f"est{s}", name=f"est{s}")
                   for s in SCALES}
            # token-major attention accum, frame pitch 128 ([s2|s4|s6|junk])
            abar = pin.tile([128, F * 128], bf16, tag="abar")
            abarT = [pin.tile([128, 384], bf16, tag=f"abarT{p}",
                              name=f"abarT{p}") for p in range(6)]

            # halting state
            ptn = pin.tile([1, F], f32, tag="ptn")
            Rt = pin.tile([1, F], f32, tag="Rt")
            wts = [pin.tile([1, F], f32, tag=f"w{it}", name=f"w{it}")
                   for it in range(2)]
            ssum = [[pin.tile([128, F], f32, tag=f"ssum{it}{c}",
                              name=f"ssum{it}{c}") for c in range(2)]
                    for it in range(2)]
            halt18 = pin.tile([18, 128], bf16, tag="halt18")
            asum_t = pin.tile([97, F], bf16, tag="asum")
            nc.vector.memset(ptn[:], 0.0)
            nc.vector.memset(Rt[:], 0.0)

            def emit_qkv(it, pieces, tagx):
                with tc.tile_pool(name=f"pq{it}{tagx}",
                                  bufs=len(pieces) + 1, space="PSUM") as ppq:
                    for gi, (wt, gt, rows) in enumerate(
                            ((wv_t, VT, 97), (wq_t, QT, 96), (wk_t, KT, 96))):
                        ptile = {}
                        for hc in range(2):
                            for p in pieces:
                                if hc == 0:
                                    ptile[p] = ppq.tile(
                                        [97, 384], f32, tag="pg", name="pg")
                                nc.tensor.matmul(
                                    ptile[p][:rows, :],
                                    wt[:, hc * rows:(hc + 1) * rows],
                                    xt[hc][p][:],
                                    start=(hc == 0), stop=(hc == 1))
                                if hc == 1:
                                    gc = p * 384
                                    if p % 2 == 0:
                                        nc.scalar.copy(
                                            gt[:, gc:gc + 384],
                                            ptile[p][:rows, :])
                                    else:
                                        nc.vector.tensor_copy(
                                            gt[:, gc:gc + 384],
                                            ptile[p][:rows, :])

            def emit_vprime(it, frames, tagx):
                with tc.tile_pool(name=f"pv{it}{tagx}", bufs=2,
                                  space="PSUM") as ppv:
                    for t in frames:
                        pvt = ppv.tile([128, 96], bf16, tag="pvt")
                        nc.tensor.transpose(
                            pvt[:], VT[0:96, t * 128:(t + 1) * 128],
                            identb[0:96, 0:96])
                        dst = vp[:, t * _VPITCH + 80:
                                 t * _VPITCH + 80 + 3 * 112]
                        nc.vector.tensor_copy(
                            dst.rearrange("p (s c) -> p s c", c=112)
                            [:, :, 0:32],
                            pvt[:].rearrange("p (s c) -> p s c", c=32))

            def emit_scores(it, fmax, tagx):
                # strips whose query range ends at frame <= fmax
                with tc.tile_pool(name=f"ps{it}{tagx}", bufs=2,
                                  space="PSUM") as pps:
                    for si, s in enumerate(SCALES):
                        offs, rng, _tot = meta[s]
                        for f2 in range(F):
                            a, b = rng[f2]
                            if not (b <= fmax if tagx == "a" else b > fmax):
                                continue
                            ncols = (b - a + 1) * 128
                            pstr = pps.tile([128, 11 * 128], f32, tag="pstr")
                            off = 0
                            for w_cc in _chunks(ncols):
                                nc.tensor.matmul(
                                    pstr[:, off:off + w_cc],
                                    KT[32 * si:32 * (si + 1),
                                       f2 * 128:(f2 + 1) * 128],
                                    QT[32 * si:32 * (si + 1),
                                       a * 128 + off:a * 128 + off + w_cc],
                                    start=True, stop=True)
                                off += w_cc
                            nc.scalar.activation(
                                est[s][:, offs[f2]:offs[f2] + ncols],
                                pstr[:, :ncols], AF.Exp)

            def emit_halting(it):
                # moved ahead of the A@V phase: only depends on halt18, and
                # its weights gate nothing until the final combine.
                with tc.tile_pool(name=f"ph{it}", bufs=1, space="PSUM") as pph:
                    elog = work.tile([18, 128], f32, tag="elog")
                    nc.scalar.activation(elog[:], halt18[:],
                                         AF.Exp, bias=nhb[:], scale=-1.0)
                    nc.vector.tensor_scalar_add(out=elog[:], in0=elog[:],
                                                scalar1=1.0)
                    sig = work.tile([18, 128], f32, tag="sig")
                    nc.vector.reciprocal(sig[:], elog[:])
                    pred = work.tile([18, 1], f32, tag="pred")
                    nc.vector.tensor_reduce(out=pred[:], in_=sig[:],
                                            axis=mybir.AxisListType.X,
                                            op=ALU.add)
                    ptp = pph.tile([1, F], f32, tag="pt")
                    nc.tensor.transpose(ptp[:], pred[:], ident[0:18, 0:18])
                    p_t = work.tile([1, F], f32, tag="p_t")
                    nc.vector.tensor_scalar_mul(out=p_t[:], in0=ptp[:],
                                                scalar1=1.0 / 128.0)

                    # halting state updates (elementwise on [1,F])
                    run_in = work.tile([1, F], f32, tag="run_in")
                    tmp = work.tile([1, F], f32, tag="tmp")
                    tmp2 = work.tile([1, F], f32, tag="tmp2")
                    nh = work.tile([1, F], f32, tag="nh")
                    run = work.tile([1, F], f32, tag="run")
                    nc.vector.tensor_scalar(out=run_in[:], in0=ptn[:],
                                            scalar1=1.0, scalar2=None,
                                            op0=ALU.is_lt)
                    nc.vector.tensor_tensor(out=tmp[:], in0=p_t[:],
                                            in1=run_in[:], op=ALU.mult)
                    nc.vector.tensor_tensor(out=tmp2[:], in0=ptn[:],
                                            in1=tmp[:], op=ALU.add)
                    nc.vector.tensor_scalar(out=tmp2[:], in0=tmp2[:],
                                            scalar1=0.99, scalar2=None,
                                            op0=ALU.is_gt)
                    nc.vector.tensor_tensor(out=nh[:], in0=tmp2[:],
                                            in1=run_in[:], op=ALU.mult)
                    nc.vector.tensor_tensor(out=run[:], in0=run_in[:],
                                            in1=nh[:], op=ALU.subtract)
                    nc.vector.tensor_tensor(out=tmp[:], in0=p_t[:],
                                            in1=run[:], op=ALU.mult)
                    nc.vector.tensor_tensor(out=ptn[:], in0=ptn[:],
                                            in1=tmp[:], op=ALU.add)
                    nc.vector.tensor_scalar(out=tmp2[:], in0=ptn[:],
                                            scalar1=-1.0, scalar2=1.0,
                                            op0=ALU.mult, op1=ALU.add)
                    nc.vector.tensor_tensor(out=tmp2[:], in0=nh[:],
                                            in1=tmp2[:], op=ALU.mult)
                    nc.vector.tensor_tensor(out=Rt[:], in0=Rt[:],
                                            in1=tmp2[:], op=ALU.add)
                    nc.vector.tensor_tensor(out=tmp2[:], in0=nh[:],
                                            in1=Rt[:], op=ALU.mult)
                    nc.vector.tensor_tensor(out=ptn[:], in0=ptn[:],
                                            in1=tmp2[:], op=ALU.add)
                    nc.vector.tensor_tensor(out=wts[it][:], in0=tmp[:],
                                            in1=tmp2[:], op=ALU.add)
                    if it == 1:
                        # context-mix coefficients; overlaps iter-2 scores
                        w1, w2 = wts[0], wts[1]
                        c1 = work.tile([1, 2 * F], f32, tag="coef")
                        nc.vector.tensor_copy(c1[:, 0:F], w2[:])
                        tmpc = work.tile([1, F], f32, tag="tmpc")
                        nc.vector.tensor_scalar(out=tmpc[:], in0=w2[:],
                                                scalar1=-1.0, scalar2=1.0,
                                                op0=ALU.mult, op1=ALU.add)
                        nc.vector.tensor_tensor(out=tmpc[:], in0=tmpc[:],
                                                in1=w1[:], op=ALU.mult)
                        nc.vector.tensor_scalar_mul(out=c1[:, F:2 * F],
                                                    in0=tmpc[:],
                                                    scalar1=1.0 / NCORES)
                        pcc = pph.tile([128, 2 * F], f32, tag="pc")
                        nc.tensor.matmul(pcc[:], ones_row[:], c1[:],
                                         start=True, stop=True)
                        nc.vector.tensor_copy(coefb[:], pcc[:])

            coefb = pin.tile([128, 2 * F], f32, tag="coefb")

            for it in range(2):
                def cb_chunk(plo, phi):
                    # AllReduce copy-backs on the ACT queue: its next work
                    # (the QKV drains) waits on this data anyway, so no
                    # engine loses issue slots behind the collective.
                    for p in range(plo, phi):
                        for hc in range(2):
                            nc.scalar.dma_start(out=xt[hc][p][:],
                                                in_=ar_out[p, hc])
                    for p in range(plo, phi):
                        for hc in range(2):
                            nc.vector.tensor_reduce(
                                out=ssum[0][hc][:, 3 * p:3 * p + 3],
                                in_=xt[hc][p][:].rearrange(
                                    "q (f c) -> q f c", c=128),
                                axis=mybir.AxisListType.X, op=ALU.add)

                if it == 0:
                    emit_qkv(0, range(6), "a")
                else:
                    # iter-2 head split by AR chunk: pieces 0-2 (chunk 1)
                    # first, so their QKV/V'/scores hide the second AllReduce
                    # instead of head-blocking the PE queue behind it.
                    cb_chunk(0, 3)
                    emit_qkv(1, range(3), "a")
                    emit_vprime(1, range(9), "a")
                    emit_scores(1, 8, "a")
                    cb_chunk(3, 6)
                    emit_qkv(1, range(3, 6), "b")

                nc.sync.dma_start(out=hbounce[:].rearrange("f p -> (f p)"),
                                  in_=VT[96:97, :])
                nc.sync.dma_start(out=halt18[:], in_=hbounce[:])

                if it == 0:
                    emit_vprime(0, range(F), "a")
                else:
                    emit_vprime(1, range(9, F), "b")

                # token-major accum cleared per iteration; col 96 of
                # each frame block is a constant 1 that becomes abarT row
                # 96 / asum row 96, turning the Wo bias into a matmul row.
                with nc.allow_low_precision(reason="bf16 abar, 2e-2 gate"):
                    nc.gpsimd.memset(abar[:], 0.0)
                    nc.gpsimd.memset(
                        abar[:].rearrange("p (f c) -> p f c", c=128)
                        [:, :, 96:97], 1.0)

                emit_scores(it, 8 if it == 1 else -1, "b")
                emit_halting(it)

                # (former skew-absorbing barrier collectives removed:
                # AR1 now fires mid-s=6-sweep with natural slack, and
                # barrier-syncing all 8 cores aligns their dense phases,
                # worsening the chip-level power throttle.)

                # ============ A@V + overlap-add ============
                with (
                    tc.tile_pool(name=f"pa{it}", bufs=2, space="PSUM") as ppa,
                    tc.tile_pool(name=f"pt{it}", bufs=2, space="PSUM") as ppt,
                    tc.tile_pool(name=f"pw{it}", bufs=2, space="PSUM") as ppw,
                ):
                    if it == 1:
                        asum_ps = ppw.tile([97, F], f32, tag="asum_ps",
                                           name="asum_ps", bufs=1)

                    def drain_group(si, s, g, gw, gtile):
                        sP = s * 128
                        av = work.tile([99, 768], bf16, tag="av_sb",
                                       bufs=3, name="av_sb")
                        # s=2/s=4 drains overlap the exp phase (ACT busy)
                        # -> vector; s=6 drains run after exp is done ->
                        # scalar, freeing vector for the rescale tail.
                        if si == 2:
                            nc.scalar.copy(av[:, :sP], gtile[:, :sP])
                        else:
                            nc.vector.tensor_copy(av[:, :sP], gtile[:, :sP])
                        ptg = ppt.tile([128, 6 * 132], bf16, tag="ptT")
                        for qc in range(s):
                            nc.tensor.transpose(
                                ptg[:, qc * 132:qc * 132 + 99],
                                av[:, qc * 128:(qc + 1) * 128],
                                identb[0:99, 0:99])
                        ptv = ptg[:, :s * 132].rearrange(
                            "p (q w v) -> p q w v", w=4, v=33)
                        rcp = work.tile([128, 18], f32, tag="rcp")
                        rcpv = rcp[:, :s * gw].rearrange(
                            "p (q w) -> p q w", w=gw)
                        nc.vector.reciprocal(rcpv, ptv[:, :, :gw, 0])
                        ab = abar[:].rearrange("p (f c) -> p f c", c=128)
                        for qc in range(s):
                            resc = work.tile([128, 96], f32, tag="resc")
                            rv = resc[:].rearrange("p (w c) -> p w c", c=32)
                            nc.vector.tensor_tensor(
                                out=rv[:, :gw, :], in0=ptv[:, qc, :gw, 1:33],
                                in1=rcpv[:, qc, :gw].broadcast_to(
                                    (128, gw, 32)),
                                op=ALU.mult)
                            with nc.allow_low_precision(
                                    reason="bf16 abar, 2e-2 gate"):
                                # accumulate on gpsimd (SBUF-only op): it
                                # unclogs the vector queue so the piece
                                # chains reach the collectives sooner.
                                nc.gpsimd.tensor_tensor(
                                    out=ab[:, 3 * g + qc:3 * g + qc + gw,
                                           si * 32:(si + 1) * 32],
                                    in0=ab[:, 3 * g + qc:3 * g + qc + gw,
                                           si * 32:(si + 1) * 32],
                                    in1=rv[:, :gw, :], op=ALU.add)

                    def frame_done(f):
                        # alternate the two hardware DMA queues (SP/ACT):
                        # serialized on one ring these 18 transposes cost
                        # ~22us on the piece->AllReduce critical chain.
                        p, fo = f // 3, f % 3
                        eng = nc.sync if f % 2 == 0 else nc.scalar
                        eng.dma_start_transpose(
                            out=abarT[p][:, fo * 128:(fo + 1) * 128],
                            in_=abar[:].rearrange("p (f c) -> p f c", c=128)
                            [:, f, :])

                    def piece_done(p):
                        # fold 1/overlap-counts in, project through Wo,
                        # stage for the chunked AllReduce.  The elementwise
                        # steps run on gpsimd/scalar so the busy vector
                        # queue never delays the collective's inputs.
                        abt = abarT[p][0:96, :].rearrange(
                            "q (f c) -> q f c", c=128)
                        cv = cinvT[:, 3 * p:3 * p + 3].broadcast_to(
                            (96, 3, 128))
                        with nc.allow_low_precision(
                                reason="bf16 abarT, 2e-2 gate"):
                            nc.gpsimd.tensor_tensor(out=abt, in0=abt, in1=cv,
                                                    op=ALU.mult)
                        for hc in range(2):
                            pw = ppw.tile([128, 384], f32, tag="pw")
                            nc.tensor.matmul(
                                pw[:], wo_t[:, hc * 128:(hc + 1) * 128],
                                abarT[p][0:97, :], start=True, stop=True)
                            with nc.allow_low_precision(
                                    reason="bf16 xt, 2e-2 gate"):
                                nc.scalar.activation(
                                    xt[hc][p][:], pw[:], AF.Copy,
                                    scale=0.25)
                            nc.sync.dma_start(out=ar_in[p, hc],
                                              in_=xt[hc][p][:])

                    def ar_chunk(plo, phi):
                        # trigger only; the copy-backs are issued from the
                        # ACT queue in the iter-2 head, right before the QKV
                        # drains that need the data, so no other queue ever
                        # stalls behind the collective.
                        nc.gpsimd.collective_compute(
                            "AllReduce", ALU.add,
                            ins=[ar_in[plo:phi]], outs=[ar_out[plo:phi]],
                            replica_groups=[list(range(NCORES))])

                    for si, s in enumerate(SCALES):
                        nw = F - s + 1
                        sP = s * 128
                        offs, rng, _tot = meta[s]
                        ngrp = (nw + 2) // 3

                        def qoff(w, j, offs=offs, rng=rng):
                            return offs[j] + (w - rng[j][0]) * 128

                        for g in range(ngrp):
                            gw = min(3, nw - 3 * g)
                            gtile = ppa.tile([99, 768], f32, tag="grp",
                                             name="grp")
                            for wi in range(gw):
                                w = 3 * g + wi
                                for j in range(w, w + s):
                                    base = qoff(w, j)
                                    st = (wi == 0 and j == w)
                                    sp = (wi == gw - 1 and j == w + s - 1)
                                    lo = (j * _VPITCH + 79 +
                                          112 * si - 33 * wi)
                                    lhs = vp[:, lo:lo + 99]
                                    off = 0
                                    for w_cc in _chunks(sP):
                                        nc.tensor.matmul(
                                            gtile[0:99, off:off + w_cc],
                                            lhs,
                                            est[s][:, base + off:
                                                   base + off + w_cc],
                                            start=st, stop=sp)
                                        off += w_cc
                            drain_group(si, s, g, gw, gtile)
                            if si == 2:
                                lof = 3 * g
                                hif = 3 * g + 3 if g < ngrp - 1 else F
                                if it == 0:
                                    for f in range(lof, hif):
                                        frame_done(f)
                                    # piece_done delayed one group so its
                                    # PE matmuls never head-block the next
                                    # group's A@V work while waiting on the
                                    # drain/transpose chain.
                                    if g > 0:
                                        piece_done(g - 1)
                                    if g == 3:
                                        ar_chunk(0, 3)
                                    if g == ngrp - 1:
                                        piece_done(g)
                                        piece_done(5)
                                        ar_chunk(3, 6)
                                else:
                                    if g > 0:
                                        for f in range(3 * g - 3, 3 * g):
                                            nc.tensor.matmul(
                                                asum_ps[0:97, f:f + 1],
                                                abar[:, f * 128:
                                                     f * 128 + 97],
                                                onesb[:], start=True,
                                                stop=True)
                                    if g == ngrp - 1:
                                        for f in range(3 * g - 3 + 3, F):
                                            nc.tensor.matmul(
                                                asum_ps[0:97, f:f + 1],
                                                abar[:, f * 128:
                                                     f * 128 + 97],
                                                onesb[:], start=True,
                                                stop=True)

                    if it == 1:
                        asum = asum_t
                        with nc.allow_low_precision(
                                reason="bf16 frame sums, 2e-2 gate"):
                            nc.vector.tensor_copy(asum[:], asum_ps[:])
                            nc.vector.tensor_tensor(out=asum[0:96, :],
                                                    in0=asum[0:96, :],
                                                    in1=cinvT[:],
                                                    op=ALU.mult)
                        for hc in range(2):
                            ps2 = ppw.tile([128, F], f32, tag="ps2",
                                           name="ps2", bufs=1)
                            nc.tensor.matmul(
                                ps2[:], wo_t[:, hc * 128:(hc + 1) * 128],
                                asum[:], start=True, stop=True)
                            nc.scalar.activation(
                                ssum[1][hc][:], ps2[:], AF.Copy,
                                scale=0.25)

            # ============ final combine (per-core, then output AR) ========
            with tc.tile_pool(name="pf", bufs=1, space="PSUM") as ppf:
                vfull = [work.tile([128, 1], f32, tag=f"vfull{hc}",
                                   name=f"vfull{hc}") for hc in range(2)]
                ob = work.tile([1, NCLS], f32, tag="ob")
                for hc in range(2):
                    t2 = work.tile([128, F], f32, tag="t2")
                    nc.vector.tensor_tensor(out=t2[:], in0=ssum[1][hc][:],
                                            in1=coefb[:, 0:F], op=ALU.mult)
                    t1 = work.tile([128, F], f32, tag="t1")
                    nc.vector.tensor_tensor(out=t1[:], in0=ssum[0][hc][:],
                                            in1=coefb[:, F:2 * F], op=ALU.mult)
                    nc.vector.tensor_tensor(out=t2[:], in0=t2[:], in1=t1[:],
                                            op=ALU.add)
                    nc.vector.tensor_reduce(out=vfull[hc][:], in_=t2[:],
                                            axis=mybir.AxisListType.X,
                                            op=ALU.add)
                off = 0
                for w_cc in _chunks(NCLS):
                    pcls = ppf.tile([1, 512], f32, tag="pcls")
                    for hc in range(2):
                        nc.tensor.matmul(
                            pcls[:, :w_cc], vfull[hc][:],
                            clsw[:, hc * NCLS + off:hc * NCLS + off + w_cc],
                            start=(hc == 0), stop=(hc == 1))
                    # clsb is pre-scaled by 1/NCORES host-side; the output
                    # AllReduce restores it.
                    nc.vector.tensor_tensor(out=ob[:, off:off + w_cc],
                                            in0=pcls[:, :w_cc],
                                            in1=clsb[:, off:off + w_cc],
                                            op=ALU.add)
                    off += w_cc
                nc.sync.dma_start(out=ar2_in[:], in_=ob[:])
                nc.gpsimd.collective_compute(
                    "AllReduce", ALU.add,
                    ins=[ar2_in[:]], outs=[ar2_out[:]],
                    replica_groups=[list(range(NCORES))])
                ob2 = work.tile([1, NCLS], f32, tag="ob2")
                nc.sync.dma_start(out=ob2[:], in_=ar2_out[:])
                nc.sync.dma_start(out=out_d[:], in_=ob2[:])

    nc.compile()
    return nc


_NC_CACHE = None


def _get_nc():
    global _NC_CACHE
    if _NC_CACHE is None:
        _NC_CACHE = build()
    return _NC_CACHE


def _prep_in_maps(inputs):
    emb = np.ascontiguousarray(np.asarray(inputs["multiscale_embed"], np.float32))
    halt_W = np.asarray(inputs["halt_W"], np.float32)
    halt_b = np.asarray(inputs["halt_b"], np.float32)
    cls_W = np.asarray(inputs["cls_W"], np.float32)
    cls_b = np.asarray(inputs["cls_b"], np.float32)
    Wq = np.asarray(inputs["mhsa_Wq"], np.float32)
    Wk = np.asarray(inputs["mhsa_Wk"], np.float32)
    Wv = np.asarray(inputs["mhsa_Wv"], np.float32)
    Wo = np.asarray(inputs["mhsa_Wo"], np.float32)
    bo = np.asarray(inputs["mhsa_bo"], np.float32)

    import ml_dtypes
    bf = ml_dtypes.bfloat16
    xt = np.ascontiguousarray(
        emb.reshape(NTOK, H).T.reshape(2, 128, NTOK)).astype(bf)
    bosum = bo.sum(axis=0)[None, :]  # [1, 256]; Wo row 96
    hwc = halt_W.reshape(256)
    nhb = np.full((18, 1), -float(halt_b[0]), np.float32)
    cinvT = np.concatenate(
        [np.repeat((1.0 / _counts(s))[None, :], 32, axis=0) for s in SCALES]
    ).astype(np.float32)  # [96, F]
    clsw = np.ascontiguousarray(cls_W.reshape(2, 128, NCLS))
    # the final combine adds clsb per-core before the output AllReduce
    clsb = (cls_b.reshape(1, NCLS) / NCORES).astype(np.float32)
    ident = np.eye(128, dtype=np.float32)
    inv_sqrt_hd = 1.0 / np.sqrt(np.float32(HD))

    in_maps = []
    for h in range(NCORES):
        sl = slice(h * HD, (h + 1) * HD)
        # grouped weights: cols = [scale2 | scale4 | scale6] head-slices
        wq_g = np.concatenate([Wq[si][:, sl] for si in range(3)], axis=1)
        wk_g = np.concatenate([Wk[si][:, sl] * inv_sqrt_hd for si in range(3)],
                              axis=1)
        wv_g = np.concatenate([Wv[si][:, sl] for si in range(3)] +
                              [hwc[:, None]], axis=1)  # [256, 97]
        wo_g = np.concatenate([Wo[si][sl, :] for si in range(3)] +
                              [bosum], axis=0)  # [97, 256]
        in_maps.append({
            "xt": xt,
            "wq": np.ascontiguousarray(wq_g.reshape(2, 128, 96)).astype(bf),
            "wk": np.ascontiguousarray(wk_g.reshape(2, 128, 96)).astype(bf),
            "wv": np.ascontiguousarray(wv_g.reshape(2, 128, 97)).astype(bf),
            "wo": np.ascontiguousarray(wo_g).astype(bf),
            "nhb": nhb, "cinvT": cinvT, "clsw": clsw,
            "clsb": clsb, "ident": ident,
        })
    return in_maps


def run(inputs, trace=False):
    _install_ntff_hook()
    # NOTE: _enable_ldw_opt() stays off: walrus's LDW dedup rejects this
    # kernel's LDWEIGHTS mix ("InstLdweights is not compatible with LDW
    # optimization"), with or without DoubleRow.
    from concourse.bass_utils import run_bass_kernel_spmd

    nc = _get_nc()
    in_maps = _prep_in_maps(inputs)
    res = run_bass_kernel_spmd(nc, in_maps, list(range(NCORES)), trace=trace)
    out = np.asarray(res.results[0]["out"], np.float32)
    return out, res


def kernel(**inputs):
    out, _ = run(inputs, trace=False)
    return out


# revision 50
# speedup vs baseline: 1.1492x; 1.0433x over previous
"""Trainium2 Bass kernel for nn_CRF_SelfAttention_65627100283470.

Math (validated vs the reference at 1e-6 rel err):
  - The CRF/marginal branch is dead code: softmax over the class dim sums
    to 1, so sum(cluster_features, 0) == sum of context rows.  The output
    is (sum_{f,p} context2) @ cls_W + cls_b.
  - context2 = w2*T2 + w1*(1-w2)*T1 with T_it the per-iteration temporal
    tensors, and w_it per-frame halting weights -> only per-frame sums of
    temporal are needed at the end.
  - QKV projections are shared across overlapping windows; exp(scores)
    strips are shared across windows (computed per key-frame strip); the
    output projection commutes with overlap-add; softmax denominators come
    from a ones-column prepended to V.
  - The 1/overlap-count scaling commutes with the A@V drain (it is a
    per-(scale, query-frame) scalar), so it is applied once per piece to
    abarT (iter 1) / once to asum (iter 2) instead of per window.

Sharding: 8 heads -> 8 cores.  The mid-kernel temporal exchange is TWO
chunked AllReduces (pieces 0-2 fired mid-s=6-sweep, pieces 3-5 at the
end) whose copy-backs issue from the ACT queue right before the iter-2
QKV that consumes them; iter-2's head is emitted in two piece-halves so
chunk-1 work hides chunk-2's collective.  The final collective reduces
the [1,625] class output itself (coefs/ssum0 are replicated).

This revision (vs the 383us v1), all bf16 on the PE:
  - abar is bf16 with a constant-ones column per frame; Wo is a 97-row
    matrix whose last row carries bo.sum, folding the bias into the
    matmul so both Wo drains are single scaled copies on ACT.
  - Wo projection + AllReduce staging run per 3-frame piece, pipelined
    one group behind the s=6 A@V sweep (PE never head-blocks on the
    drain/transpose chain).
  - iter-2 frame sums via tiny PE column-sum matmuls (no DMA transposes)
    and the halting/coef chains hoisted to the iteration head.
  - drain: one batched reciprocal per group (132-pitch transposed
    tiles); overlap-add accumulation on gpsimd; overlap-count scaling
    applied once per piece (abarT) / once to asum instead of per window.

Measured dead ends (do not revisit without new evidence): fp8e4
DoubleRow A@V is ~30% SLOWER end-to-end on this stack despite fewer
billed PE columns (and full-fp8 est+vp fails the 2e-2 gate at 2.7e-2;
s=6-only passes at 1.4e-2 but costs +106us); walrus --enable-ldw-opt
rejects this kernel's LDWEIGHTS mix; interleaving score strips into the
A@V sweep fragments the PE stream and loses ~20us to p-state/HAM;
6 per-piece AllReduces serialize on ~10-25us/collective rendezvous.
"""
import sys
import types

import numpy as np

F, P, H, HEADS, C, NCLS = 18, 128, 256, 8, 32, 625
SCALES = (2, 4, 6)
HD = H // HEADS
NTOK = F * P  # 2304
NCORES = 8

# frame block pitch inside vp (ones+V layout with zero guards)
_VPITCH = 352  # 16-aligned V blocks: ones at 79+112*si, V at 80+112*si


def _enable_ldw_opt():
    """Walrus's LDWEIGHTS dedup is disabled by default in bass_utils;
    enable it (verified numerically by the rel-err gate in test.py)."""
    import concourse.bass_utils as bu

    if getattr(bu, "_ldw_opt_patched", False):
        return
    orig = bu.bir_verify_and_optimise

    def patched(*args, **kwargs):
        real_run = bu.run_command

        def run_hook(argv, **kw):
            argv = ["--enable-ldw-opt=true" if a == "--enable-ldw-opt=false"
                    else a for a in argv]
            return real_run(argv, **kw)

        bu.run_command = run_hook
        try:
            return orig(*args, **kwargs)
        finally:
            bu.run_command = real_run

    bu.bir_verify_and_optimise = patched
    bu._ldw_opt_patched = True


def _install_ntff_hook():
    """Recreate the missing antenv.axon_hooks so trace=True works."""
    if "antenv.axon_hooks" in sys.modules:
        return
    try:
        import antenv

        mod = types.ModuleType("antenv.axon_hooks")
        mod._hook = None
        mod.set_axon_ntff_profile_hook = lambda h: setattr(mod, "_hook", h)
        mod.get_axon_ntff_profile_hook = lambda: mod._hook
        sys.modules["antenv.axon_hooks"] = mod
        antenv.axon_hooks = mod
        from trn_agent_boot.trn_boot import _ntff_profile_via_ctypes

        mod.set_axon_ntff_profile_hook(
            _ntff_profile_via_ctypes("/opt/axon/libaxon_pjrt.so")
        )
    except Exception:
        pass


def _chunks(n, lim=512):
    out = [lim] * (n // lim)
    if n % lim:
        out.append(n % lim)
    return out


def _counts(s):
    nw = F - s + 1
    c = np.zeros(F, np.float32)
    for w in range(nw):
        c[w:w + s] += 1.0
    return c


def _strip_meta(s):
    """Per key-frame strip [a, b] ranges and col offsets in the est tile."""
    offs, rng = [], []
    off = 0
    for f2 in range(F):
        a = max(0, f2 - s + 1)
        b = min(F - 1, f2 + s - 1)
        offs.append(off)
        rng.append((a, b))
        off += (b - a + 1) * 128
    return offs, rng, off


def _est_slack(s, meta):
    """Extra est cols needed so the [p, 2, D] pair-view stays in bounds."""
    offs, rng, tot = meta
    nw = F - s + 1
    slack = 0

    def qoff(w, j):
        return offs[j] + (w - rng[j][0]) * 128

    for w in range(nw):
        for t in range(s // 2):
            j0 = w + 2 * t
            base = qoff(w, j0)
            D = qoff(w, j0 + 1) - base
            off = 0
            for ck in _chunks(s * 128):
                assert D >= ck, (s, w, t, D, ck)
                slack = max(slack, base + off + 2 * D - tot)
                off += ck
    return slack


def build():
    import concourse.bacc as bacc
    import concourse.mybir as mybir
    from concourse.tile import TileContext

    dt = mybir.dt
    f32 = dt.float32
    bf16 = dt.bfloat16
    f8 = dt.float8e4
    AF = mybir.ActivationFunctionType
    ALU = mybir.AluOpType
    DR = mybir.MatmulPerfMode.DoubleRow

    nc = bacc.Bacc("TRN2", target_bir_lowering=False, debug=False,
                   num_devices=NCORES)

    # ---- I/O ----
    xt_in = nc.dram_tensor("xt", [2, 128, NTOK], bf16, kind="ExternalInput")
    wq_in = nc.dram_tensor("wq", [2, 128, 96], bf16, kind="ExternalInput")
    wk_in = nc.dram_tensor("wk", [2, 128, 96], bf16, kind="ExternalInput")
    wv_in = nc.dram_tensor("wv", [2, 128, 97], bf16, kind="ExternalInput")
    # row 96 of wo carries bo.sum (bias folded into the matmul via the
    # constant-ones column of abar / asum)
    wo_in = nc.dram_tensor("wo", [97, 256], bf16, kind="ExternalInput")
    nhb_in = nc.dram_tensor("nhb", [18, 1], f32, kind="ExternalInput")
    cinvT_in = nc.dram_tensor("cinvT", [96, F], f32, kind="ExternalInput")
    clsw_in = nc.dram_tensor("clsw", [2, 128, NCLS], f32, kind="ExternalInput")
    clsb_in = nc.dram_tensor("clsb", [1, NCLS], f32, kind="ExternalInput")
    id_in = nc.dram_tensor("ident", [128, 128], f32, kind="ExternalInput")
    out_d = nc.dram_tensor("out", [1, NCLS], f32, kind="ExternalOutput")

    # piece-major AllReduce bounce buffers: [piece, half, 128, 384]
    ar_in = nc.dram_tensor("ar_in", [6, 2, 128, 384], bf16)
    ar_out = nc.dram_tensor("ar_out", [6, 2, 128, 384], bf16,
                            addr_space="Shared")
    bar_in = nc.dram_tensor("bar_in", [1, 1], f32)
    bar_out = nc.dram_tensor("bar_out", [1, 1], f32, addr_space="Shared")
    ar2_in = nc.dram_tensor("ar2_in", [1, NCLS], f32)
    ar2_out = nc.dram_tensor("ar2_out", [1, NCLS], f32, addr_space="Shared")
    hbounce = nc.dram_tensor("hbounce", [18, 128], bf16)

    col_cc = _chunks(NTOK)  # [512]*4 + [256]
    meta = {s: _strip_meta(s) for s in SCALES}
    slack = {s: _est_slack(s, meta[s]) for s in SCALES}

    with TileContext(nc) as tc:
        with (
            tc.tile_pool(name="pin", bufs=1) as pin,
            tc.tile_pool(name="work", bufs=3) as work,
        ):
            # ---- persistent tiles + weight loads ----
            xt = [[pin.tile([128, 384], bf16, tag=f"xt{c}{p}",
                            name=f"xt{c}{p}") for p in range(6)]
                  for c in range(2)]
            wq_t = pin.tile([128, 2 * 96], bf16, tag="wq")
            wk_t = pin.tile([128, 2 * 96], bf16, tag="wk")
            wv_t = pin.tile([128, 2 * 97], bf16, tag="wv")
            wo_t = pin.tile([97, 256], bf16, tag="wo")
            nhb = pin.tile([18, 1], f32, tag="nhb")
            cinvT = pin.tile([96, F], f32, tag="cinvT")
            clsw = pin.tile([128, 2 * NCLS], f32, tag="clsw")
            clsb = pin.tile([1, NCLS], f32, tag="clsb")
            ident = pin.tile([128, 128], f32, tag="ident")
            identb = pin.tile([128, 128], bf16, tag="identb")
            ones_row = pin.tile([1, 128], f32, tag="ones_row")
            onesb = pin.tile([128, 1], bf16, tag="onesb")

            for c in range(2):
                for p in range(6):
                    eng = nc.sync if p % 2 == 0 else nc.gpsimd
                    eng.dma_start(out=xt[c][p][:],
                                  in_=xt_in[c, :, p * 384:(p + 1) * 384])
                nc.gpsimd.dma_start(out=wq_t[:, c * 96:(c + 1) * 96],
                                    in_=wq_in[c])
                nc.gpsimd.dma_start(out=wk_t[:, c * 96:(c + 1) * 96],
                                    in_=wk_in[c])
                nc.sync.dma_start(out=wv_t[:, c * 97:(c + 1) * 97], in_=wv_in[c])
                nc.gpsimd.dma_start(out=clsw[:, c * NCLS:(c + 1) * NCLS],
                                    in_=clsw_in[c])
            nc.gpsimd.dma_start(out=wo_t[:], in_=wo_in[:])
            nc.sync.dma_start(out=nhb[:], in_=nhb_in[:])
            nc.sync.dma_start(out=cinvT[:], in_=cinvT_in[:])
            nc.gpsimd.dma_start(out=clsb[:], in_=clsb_in[:])
            nc.gpsimd.dma_start(out=ident[:], in_=id_in[:])
            nc.vector.memset(ones_row[:], 1.0)
            nc.vector.tensor_copy(identb[:], ident[:])
            with nc.allow_low_precision(reason="bf16 ones, exact"):
                nc.vector.memset(onesb[:], 1.0)

            # grouped projections (token cols)
            QT = pin.tile([96, NTOK], bf16, tag="QT")
            KT = pin.tile([96, NTOK], bf16, tag="KT")
            VT = pin.tile([97, NTOK], bf16, tag="VT")
            # V' tile: per frame [zeros | 1 | V(3 scales) | zeros]
            vp = pin.tile([128, F * _VPITCH + 64], bf16, tag="vp")
            nc.vector.memset(vp[:], 0.0)
            for si in range(3):
                nc.vector.memset(
                    vp[:, :F * _VPITCH].rearrange(
                        "p (f c) -> p f c", c=_VPITCH)
                    [:, :, 79 + 112 * si:80 + 112 * si], 1.0)

            # est strips per scale
            est = {s: pin.tile([128, meta[s][2]], bf16,
                               tag=f"est{s}", name=f"est{s}")
                   for s in SCALES}
            # token-major attention accum, frame pitch 128 ([s2|s4|s6|junk])
            abar = pin.tile([128, F * 128], bf16, tag="abar")
            abarT = [pin.tile([128, 384], bf16, tag=f"abarT{p}",
                              name=f"abarT{p}") for p in range(6)]

            # halting state
            ptn = pin.tile([1, F], f32, tag="ptn")
            Rt = pin.tile([1, F], f32, tag="Rt")
            wts = [pin.tile([1, F], f32, tag=f"w{it}", name=f"w{it}")
                   for it in range(2)]
            ssum = [[pin.tile([128, F], f32, tag=f"ssum{it}{c}",
                              name=f"ssum{it}{c}") for c in range(2)]
                    for it in range(2)]
            halt18 = pin.tile([18, 128], bf16, tag="halt18")
            asum_t = pin.tile([97, F], bf16, tag="asum")
            nc.vector.memset(ptn[:], 0.0)
            nc.vector.memset(Rt[:], 0.0)

            def emit_qkv(it, pieces, tagx):
                with tc.tile_pool(name=f"pq{it}{tagx}",
                                  bufs=len(pieces) + 1, space="PSUM") as ppq:
                    for gi, (wt, gt, rows) in enumerate(
                            ((wv_t, VT, 97), (wq_t, QT, 96), (wk_t, KT, 96))):
                        ptile = {}
                        for hc in range(2):
                            for p in pieces:
                                if hc == 0:
                                    ptile[p] = ppq.tile(
                                        [97, 384], f32, tag="pg", name="pg")
                                nc.tensor.matmul(
                                    ptile[p][:rows, :],
                                    wt[:, hc * rows:(hc + 1) * rows],
                                    xt[hc][p][:],
                                    start=(hc == 0), stop=(hc == 1))
                                if hc == 1:
                                    gc = p * 384
                                    if p % 2 == 0:
                                        nc.scalar.copy(
                                            gt[:, gc:gc + 384],
                                            ptile[p][:rows, :])
                                    else:
                                        nc.vector.tensor_copy(
                                            gt[:, gc:gc + 384],
                                            ptile[p][:rows, :])

            def emit_vprime(it, frames, tagx):
                with tc.tile_pool(name=f"pv{it}{tagx}", bufs=2,
                                  space="PSUM") as ppv:
                    for t in frames:
                        pvt = ppv.tile([128, 96], bf16, tag="pvt")
                        nc.tensor.transpose(
                            pvt[:], VT[0:96, t * 128:(t + 1) * 128],
                            identb[0:96, 0:96])
                        dst = vp[:, t * _VPITCH + 80:
                                 t * _VPITCH + 80 + 3 * 112]
                        nc.vector.tensor_copy(
                            dst.rearrange("p (s c) -> p s c", c=112)
                            [:, :, 0:32],
                            pvt[:].rearrange("p (s c) -> p s c", c=32))

            def emit_scores(it, fmax, tagx):
                # strips whose query range ends at frame <= fmax
                with tc.tile_pool(name=f"ps{it}{tagx}", bufs=2,
                                  space="PSUM") as pps:
                    for si, s in enumerate(SCALES):
                        offs, rng, _tot = meta[s]
                        for f2 in range(F):
                            a, b = rng[f2]
                            if not (b <= fmax if tagx == "a" else b > fmax):
                                continue
                            ncols = (b - a + 1) * 128
                            pstr = pps.tile([128, 11 * 128], f32, tag="pstr")
                            off = 0
                            for w_cc in _chunks(ncols):
                                nc.tensor.matmul(
                                    pstr[:, off:off + w_cc],
                                    KT[32 * si:32 * (si + 1),
                                       f2 * 128:(f2 + 1) * 128],
                                    QT[32 * si:32 * (si + 1),
                                       a * 128 + off:a * 128 + off + w_cc],
                                    start=True, stop=True)
                                off += w_cc
                            nc.scalar.activation(
                                est[s][:, offs[f2]:offs[f2] + ncols],
                                pstr[:, :ncols], AF.Exp)

            def emit_halting(it):
                # moved ahead of the A@V phase: only depends on halt18, and
                # its weights gate nothing until the final combine.
                with tc.tile_pool(name=f"ph{it}", bufs=1, space="PSUM") as pph:
                    elog = work.tile([18, 128], f32, tag="elog")
                    nc.scalar.activation(elog[:], halt18[:],
                                         AF.Exp, bias=nhb[:], scale=-1.0)
                    nc.vector.tensor_scalar_add(out=elog[:], in0=elog[:],
                                                scalar1=1.0)
                    sig = work.tile([18, 128], f32, tag="sig")
                    nc.vector.reciprocal(sig[:], elog[:])
                    pred = work.tile([18, 1], f32, tag="pred")
                    nc.vector.tensor_reduce(out=pred[:], in_=sig[:],
                                            axis=mybir.AxisListType.X,
                                            op=ALU.add)
                    ptp = pph.tile([1, F], f32, tag="pt")
                    nc.tensor.transpose(ptp[:], pred[:], ident[0:18, 0:18])
                    p_t = work.tile([1, F], f32, tag="p_t")
                    nc.vector.tensor_scalar_mul(out=p_t[:], in0=ptp[:],
                                                scalar1=1.0 / 128.0)

                    # halting state updates (elementwise on [1,F])
                    run_in = work.tile([1, F], f32, tag="run_in")
                    tmp = work.tile([1, F], f32, tag="tmp")
                    tmp2 = work.tile([1, F], f32, tag="tmp2")
                    nh = work.tile([1, F], f32, tag="nh")
                    run = work.tile([1, F], f32, tag="run")
                    nc.vector.tensor_scalar(out=run_in[:], in0=ptn[:],
                                            scalar1=1.0, scalar2=None,
                                            op0=ALU.is_lt)
                    nc.vector.tensor_tensor(out=tmp[:], in0=p_t[:],
                                            in1=run_in[:], op=ALU.mult)
                    nc.vector.tensor_tensor(out=tmp2[:], in0=ptn[:],
                                            in1=tmp[:], op=ALU.add)
                    nc.vector.tensor_scalar(out=tmp2[:], in0=tmp2[:],
                                            scalar1=0.99, scalar2=None,
                                            op0=ALU.is_gt)
                    nc.vector.tensor_tensor(out=nh[:], in0=tmp2[:],
                                            in1=run_in[:], op=ALU.mult)
                    nc.vector.tensor_tensor(out=run[:], in0=run_in[:],
                                            in1=nh[:], op=ALU.subtract)
                    nc.vector.tensor_tensor(out=tmp[:], in0=p_t[:],
                                            in1=run[:], op=ALU.mult)
                    nc.vector.tensor_tensor(out=ptn[:], in0=ptn[:],
                                            in1=tmp[:], op=ALU.add)
                    nc.vector.tensor_scalar(out=tmp2[:], in0=ptn[:],
                                            scalar1=-1.0, scalar2=1.0,
                                            op0=ALU.mult, op1=ALU.add)
                    nc.vector.tensor_tensor(out=tmp2[:], in0=nh[:],
                                            in1=tmp2[:], op=ALU.mult)
                    nc.vector.tensor_tensor(out=Rt[:], in0=Rt[:],
                                            in1=tmp2[:], op=ALU.add)
                    nc.vector.tensor_tensor(out=tmp2[:], in0=nh[:],
                                            in1=Rt[:], op=ALU.mult)
                    nc.vector.tensor_tensor(out=ptn[:], in0=ptn[:],
                                            in1=tmp2[:], op=ALU.add)
                    nc.vector.tensor_tensor(out=wts[it][:], in0=tmp[:],
                                            in1=tmp2[:], op=ALU.add)
                    if it == 1:
                        # context-mix coefficients; overlaps iter-2 scores
                        w1, w2 = wts[0], wts[1]
                        c1 = work.tile([1, 2 * F], f32, tag="coef")
                        nc.vector.tensor_copy(c1[:, 0:F], w2[:])
                        tmpc = work.tile([1, F], f32, tag="tmpc")
                        nc.vector.tensor_scalar(out=tmpc[:], in0=w2[:],
                                                scalar1=-1.0, scalar2=1.0,
                                                op0=ALU.mult, op1=ALU.add)
                        nc.vector.tensor_tensor(out=tmpc[:], in0=tmpc[:],
                                                in1=w1[:], op=ALU.mult)
                        nc.vector.tensor_scalar_mul(out=c1[:, F:2 * F],
                                                    in0=tmpc[:],
                                                    scalar1=1.0 / NCORES)
                        pcc = pph.tile([128, 2 * F], f32, tag="pc")
                        nc.tensor.matmul(pcc[:], ones_row[:], c1[:],
                                         start=True, stop=True)
                        nc.vector.tensor_copy(coefb[:], pcc[:])

            coefb = pin.tile([128, 2 * F], f32, tag="coefb")

            for it in range(2):
                def cb_chunk(plo, phi):
                    # AllReduce copy-backs on the ACT queue: its next work
                    # (the QKV drains) waits on this data anyway, so no
                    # engine loses issue slots behind the collective.
                    for p in range(plo, phi):
                        for hc in range(2):
                            nc.scalar.dma_start(out=xt[hc][p][:],
                                                in_=ar_out[p, hc])
                    for p in range(plo, phi):
                        for hc in range(2):
                            nc.vector.tensor_reduce(
                                out=ssum[0][hc][:, 3 * p:3 * p + 3],
                                in_=xt[hc][p][:].rearrange(
                                    "q (f c) -> q f c", c=128),
                                axis=mybir.AxisListType.X, op=ALU.add)

                if it == 0:
                    emit_qkv(0, range(6), "a")
                else:
                    # iter-2 head split by AR chunk: pieces 0-2 (chunk 1)
                    # first, so their QKV/V'/scores hide the second AllReduce
                    # instead of head-blocking the PE queue behind it.
                    cb_chunk(0, 3)
                    emit_qkv(1, range(3), "a")
                    emit_vprime(1, range(9), "a")
                    emit_scores(1, 8, "a")
                    cb_chunk(3, 6)
                    emit_qkv(1, range(3, 6), "b")

                nc.sync.dma_start(out=hbounce[:].rearrange("f p -> (f p)"),
                                  in_=VT[96:97, :])
                nc.sync.dma_start(out=halt18[:], in_=hbounce[:])

                if it == 0:
                    emit_vprime(0, range(F), "a")
                else:
                    emit_vprime(1, range(9, F), "b")

                # token-major accum cleared per iteration; col 96 of
                # each frame block is a constant 1 that becomes abarT row
                # 96 / asum row 96, turning the Wo bias into a matmul row.
                with nc.allow_low_precision(reason="bf16 abar, 2e-2 gate"):
                    nc.gpsimd.memset(abar[:], 0.0)
                    nc.gpsimd.memset(
                        abar[:].rearrange("p (f c) -> p f c", c=128)
                        [:, :, 96:97], 1.0)

                emit_scores(it, 8 if it == 1 else -1, "b")
                emit_halting(it)

                if it == 0:
                    # skew-absorbing barriers, hidden under attention
                    nc.sync.dma_start(out=bar_in[:], in_=nhb[0:1, 0:1])
                    for _ in range(2):
                        nc.gpsimd.collective_compute(
                            "AllReduce", ALU.add,
                            ins=[bar_in[:]], outs=[bar_out[:]],
                            replica_groups=[list(range(NCORES))])

                # ============ A@V + overlap-add ============
                with (
                    tc.tile_pool(name=f"pa{it}", bufs=2, space="PSUM") as ppa,
                    tc.tile_pool(name=f"pt{it}", bufs=2, space="PSUM") as ppt,
                    tc.tile_pool(name=f"pw{it}", bufs=2, space="PSUM") as ppw,
                ):
                    if it == 1:
                        asum_ps = ppw.tile([97, F], f32, tag="asum_ps",
                                           name="asum_ps", bufs=1)

                    def drain_group(si, s, g, gw, gtile):
                        sP = s * 128
                        av = work.tile([99, 768], bf16, tag="av_sb",
                                       bufs=3, name="av_sb")
                        # s=2/s=4 drains overlap the exp phase (ACT busy)
                        # -> vector; s=6 drains run after exp is done ->
                        # scalar, freeing vector for the rescale tail.
                        if si == 2:
                            nc.scalar.copy(av[:, :sP], gtile[:, :sP])
                        else:
                            nc.vector.tensor_copy(av[:, :sP], gtile[:, :sP])
                        ptg = ppt.tile([128, 6 * 132], bf16, tag="ptT")
                        for qc in range(s):
                            nc.tensor.transpose(
                                ptg[:, qc * 132:qc * 132 + 99],
                                av[:, qc * 128:(qc + 1) * 128],
                                identb[0:99, 0:99])
                        ptv = ptg[:, :s * 132].rearrange(
                            "p (q w v) -> p q w v", w=4, v=33)
                        rcp = work.tile([128, 18], f32, tag="rcp")
                        rcpv = rcp[:, :s * gw].rearrange(
                            "p (q w) -> p q w", w=gw)
                        nc.vector.reciprocal(rcpv, ptv[:, :, :gw, 0])
                        ab = abar[:].rearrange("p (f c) -> p f c", c=128)
                        for qc in range(s):
                            resc = work.tile([128, 96], f32, tag="resc")
                            rv = resc[:].rearrange("p (w c) -> p w c", c=32)
                            nc.vector.tensor_tensor(
                                out=rv[:, :gw, :], in0=ptv[:, qc, :gw, 1:33],
                                in1=rcpv[:, qc, :gw].broadcast_to(
                                    (128, gw, 32)),
                                op=ALU.mult)
                            with nc.allow_low_precision(
                                    reason="bf16 abar, 2e-2 gate"):
                                # accumulate on gpsimd (SBUF-only op): it
                                # unclogs the vector queue so the piece
                                # chains reach the collectives sooner.
                                nc.gpsimd.tensor_tensor(
                                    out=ab[:, 3 * g + qc:3 * g + qc + gw,
                                           si * 32:(si + 1) * 32],
                                    in0=ab[:, 3 * g + qc:3 * g + qc + gw,
                                           si * 32:(si + 1) * 32],
                                    in1=rv[:, :gw, :], op=ALU.add)

                    def frame_done(f):
                        # alternate the two hardware DMA queues (SP/ACT):
                        # serialized on one ring these 18 transposes cost
                        # ~22us on the piece->AllReduce critical chain.
                        p, fo = f // 3, f % 3
                        eng = nc.sync if f % 2 == 0 else nc.scalar
                        eng.dma_start_transpose(
                            out=abarT[p][:, fo * 128:(fo + 1) * 128],
                            in_=abar[:].rearrange("p (f c) -> p f c", c=128)
                            [:, f, :])

                    def piece_done(p):
                        # fold 1/overlap-counts in, project through Wo,
                        # stage for the chunked AllReduce.  The elementwise
                        # steps run on gpsimd/scalar so the busy vector
                        # queue never delays the collective's inputs.
                        abt = abarT[p][0:96, :].rearrange(
                            "q (f c) -> q f c", c=128)
                        cv = cinvT[:, 3 * p:3 * p + 3].broadcast_to(
                            (96, 3, 128))
                        with nc.allow_low_precision(
                                reason="bf16 abarT, 2e-2 gate"):
                            nc.gpsimd.tensor_tensor(out=abt, in0=abt, in1=cv,
                                                    op=ALU.mult)
                        for hc in range(2):
                            pw = ppw.tile([128, 384], f32, tag="pw")
                            nc.tensor.matmul(
                                pw[:], wo_t[:, hc * 128:(hc + 1) * 128],
                                abarT[p][0:97, :], start=True, stop=True)
                            with nc.allow_low_precision(
                                    reason="bf16 xt, 2e-2 gate"):
                                nc.scalar.activation(
                                    xt[hc][p][:], pw[:], AF.Copy,
                                    scale=0.25)
                            nc.sync.dma_start(out=ar_in[p, hc],
                                              in_=xt[hc][p][:])

                    def ar_chunk(plo, phi):
                        # trigger only; the copy-backs are issued from the
                        # ACT queue in the iter-2 head, right before the QKV
                        # drains that need the data, so no other queue ever
                        # stalls behind the collective.
                        nc.gpsimd.collective_compute(
                            "AllReduce", ALU.add,
                            ins=[ar_in[plo:phi]], outs=[ar_out[plo:phi]],
                            replica_groups=[list(range(NCORES))])

                    for si, s in enumerate(SCALES):
                        nw = F - s + 1
                        sP = s * 128
                        offs, rng, _tot = meta[s]
                        ngrp = (nw + 2) // 3

                        def qoff(w, j, offs=offs, rng=rng):
                            return offs[j] + (w - rng[j][0]) * 128

                        for g in range(ngrp):
                            gw = min(3, nw - 3 * g)
                            gtile = ppa.tile([99, 768], f32, tag="grp",
                                             name="grp")
                            for wi in range(gw):
                                w = 3 * g + wi
                                for j in range(w, w + s):
                                    base = qoff(w, j)
                                    st = (wi == 0 and j == w)
                                    sp = (wi == gw - 1 and j == w + s - 1)
                                    lo = (j * _VPITCH + 79 +
                                          112 * si - 33 * wi)
                                    lhs = vp[:, lo:lo + 99]
                                    off = 0
                                    for w_cc in _chunks(sP):
                                        nc.tensor.matmul(
                                            gtile[0:99, off:off + w_cc],
                                            lhs,
                                            est[s][:, base + off:
                                                   base + off + w_cc],
                                            start=st, stop=sp)
                                        off += w_cc
                            drain_group(si, s, g, gw, gtile)
                            if si == 2:
                                lof = 3 * g
                                hif = 3 * g + 3 if g < ngrp - 1 else F
                                if it == 0:
                                    for f in range(lof, hif):
                                        frame_done(f)
                                    # piece_done delayed one group so its
                                    # PE matmuls never head-block the next
                                    # group's A@V work while waiting on the
                                    # drain/transpose chain.
                                    if g > 0:
                                        piece_done(g - 1)
                                    if g == 3:
                                        ar_chunk(0, 3)
                                    if g == ngrp - 1:
                                        piece_done(g)
                                        piece_done(5)
                                        ar_chunk(3, 6)
                                else:
                                    if g > 0:
                                        for f in range(3 * g - 3, 3 * g):
                                            nc.tensor.matmul(
                                                asum_ps[0:97, f:f + 1],
                                                abar[:, f * 128:
                                                     f * 128 + 97],
                                                onesb[:], start=True,
                                                stop=True)
                                    if g == ngrp - 1:
                                        for f in range(3 * g - 3 + 3, F):
                                            nc.tensor.matmul(
                                                asum_ps[0:97, f:f + 1],
                                                abar[:, f * 128:
                                                     f * 128 + 97],
                                                onesb[:], start=True,
                                                stop=True)

                    if it == 1:
                        asum = asum_t
                        with nc.allow_low_precision(
                                reason="bf16 frame sums, 2e-2 gate"):
                            nc.vector.tensor_copy(asum[:], asum_ps[:])
                            nc.vector.tensor_tensor(out=asum[0:96, :],
                                                    in0=asum[0:96, :],
                                                    in1=cinvT[:],
                                                    op=ALU.mult)
                        for hc in range(2):
                            ps2 = ppw.tile([128, F], f32, tag="ps2",
                                           name="ps2", bufs=1)
                            nc.tensor.matmul(
                                ps2[:], wo_t[:, hc * 128:(hc + 1) * 128],
                                asum[:], start=True, stop=True)
                            nc.scalar.activation(
                                ssum[1][hc][:], ps2[:], AF.Copy,
                                scale=0.25)

            # ============ final combine (per-core, then output AR) ========
            with tc.tile_pool(name="pf", bufs=1, space="PSUM") as ppf:
                vfull = [work.tile([128, 1], f32, tag=f"vfull{hc}",
                                   name=f"vfull{hc}") for hc in range(2)]
                ob = work.tile([1, NCLS], f32, tag="ob")
                for hc in range(2):
                    t2 = work.tile([128, F], f32, tag="t2")
                    nc.vector.tensor_tensor(out=t2[:], in0=ssum[1][hc][:],
                                            in1=coefb[:, 0:F], op=ALU.mult)
                    t1 = work.tile([128, F], f32, tag="t1")
                    nc.vector.tensor_tensor(out=t1[:], in0=ssum[0][hc][:],
                                            in1=coefb[:, F:2 * F], op=ALU.mult)
                    nc.vector.tensor_tensor(out=t2[:], in0=t2[:], in1=t1[:],
                                            op=ALU.add)
                    nc.vector.tensor_reduce(out=vfull[hc][:], in_=t2[:],
                                            axis=mybir.AxisListType.X,
                                            op=ALU.add)
                off = 0
                for w_cc in _chunks(NCLS):
                    pcls = ppf.tile([1, 512], f32, tag="pcls")
                    for hc in range(2):
                        nc.tensor.matmul(
                            pcls[:, :w_cc], vfull[hc][:],
                            clsw[:, hc * NCLS + off:hc * NCLS + off + w_cc],
                            start=(hc == 0), stop=(hc == 1))
                    # clsb is pre-scaled by 1/NCORES host-side; the output
                    # AllReduce restores it.
                    nc.vector.tensor_tensor(out=ob[:, off:off + w_cc],
                                            in0=pcls[:, :w_cc],
                                            in1=clsb[:, off:off + w_cc],
                                            op=ALU.add)
                    off += w_cc
                nc.sync.dma_start(out=ar2_in[:], in_=ob[:])
                nc.gpsimd.collective_compute(
                    "AllReduce", ALU.add,
                    ins=[ar2_in[:]], outs=[ar2_out[:]],
                    replica_groups=[list(range(NCORES))])
                ob2 = work.tile([1, NCLS], f32, tag="ob2")
                nc.sync.dma_start(out=ob2[:], in_=ar2_out[:])
                nc.sync.dma_start(out=out_d[:], in_=ob2[:])

    nc.compile()
    return nc


_NC_CACHE = None


def _get_nc():
    global _NC_CACHE
    if _NC_CACHE is None:
        _NC_CACHE = build()
    return _NC_CACHE


def _prep_in_maps(inputs):
    emb = np.ascontiguousarray(np.asarray(inputs["multiscale_embed"], np.float32))
    halt_W = np.asarray(inputs["halt_W"], np.float32)
    halt_b = np.asarray(inputs["halt_b"], np.float32)
    cls_W = np.asarray(inputs["cls_W"], np.float32)
    cls_b = np.asarray(inputs["cls_b"], np.float32)
    Wq = np.asarray(inputs["mhsa_Wq"], np.float32)
    Wk = np.asarray(inputs["mhsa_Wk"], np.float32)
    Wv = np.asarray(inputs["mhsa_Wv"], np.float32)
    Wo = np.asarray(inputs["mhsa_Wo"], np.float32)
    bo = np.asarray(inputs["mhsa_bo"], np.float32)

    import ml_dtypes
    bf = ml_dtypes.bfloat16
    xt = np.ascontiguousarray(
        emb.reshape(NTOK, H).T.reshape(2, 128, NTOK)).astype(bf)
    bosum = bo.sum(axis=0)[None, :]  # [1, 256]; Wo row 96
    hwc = halt_W.reshape(256)
    nhb = np.full((18, 1), -float(halt_b[0]), np.float32)
    cinvT = np.concatenate(
        [np.repeat((1.0 / _counts(s))[None, :], 32, axis=0) for s in SCALES]
    ).astype(np.float32)  # [96, F]
    clsw = np.ascontiguousarray(cls_W.reshape(2, 128, NCLS))
    # the final combine adds clsb per-core before the output AllReduce
    clsb = (cls_b.reshape(1, NCLS) / NCORES).astype(np.float32)
    ident = np.eye(128, dtype=np.float32)
    inv_sqrt_hd = 1.0 / np.sqrt(np.float32(HD))

    in_maps = []
    for h in range(NCORES):
        sl = slice(h * HD, (h + 1) * HD)
        # grouped weights: cols = [scale2 | scale4 | scale6] head-slices
        wq_g = np.concatenate([Wq[si][:, sl] for si in range(3)], axis=1)
        wk_g = np.concatenate([Wk[si][:, sl] * inv_sqrt_hd for si in range(3)],
                              axis=1)
        wv_g = np.concatenate([Wv[si][:, sl] for si in range(3)] +
                              [hwc[:, None]], axis=1)  # [256, 97]
        wo_g = np.concatenate([Wo[si][sl, :] for si in range(3)] +
                              [bosum], axis=0)  # [97, 256]
        in_maps.append({
            "xt": xt,
            "wq": np.ascontiguousarray(wq_g.reshape(2, 128, 96)).astype(bf),
            "wk": np.ascontiguousarray(wk_g.reshape(2, 128, 96)).astype(bf),
            "wv": np.ascontiguousarray(wv_g.reshape(2, 128, 97)).astype(bf),
            "wo": np.ascontiguousarray(wo_g).astype(bf),
            "nhb": nhb, "cinvT": cinvT, "clsw": clsw,
            "clsb": clsb, "ident": ident,
        })
    return in_maps


def run(inputs, trace=False):
    _install_ntff_hook()
    # NOTE: _enable_ldw_opt() stays off: walrus's LDW dedup rejects this
    # kernel's LDWEIGHTS mix ("InstLdweights is not compatible with LDW
    # optimization"), with or without DoubleRow.
    from concourse.bass_utils import run_bass_kernel_spmd

    nc = _get_nc()
    in_maps = _prep_in_maps(inputs)
    res = run_bass_kernel_spmd(nc, in_maps, list(range(NCORES)), trace=trace)
    out = np.asarray(res.results[0]["out"], np.float32)
    return out, res


def kernel(**inputs):
    out, _ = run(inputs, trace=False)
    return out
